# revision 2
# baseline (speedup 1.0000x reference)
"""Trainium2 Bass kernel for the MHSA bottleneck block.

Contract: kernel(**inputs) takes the FULL unsharded inputs (as produced by
setup_inputs()) and returns the FULL [64, 2048, 14, 14] float32 output.
Internally shards data-parallel over batch: 8 images per NeuronCore, 8 cores.
"""
import sys

sys.path.insert(0, '/opt/trn_rl_repo')

import numpy as np

# Problem constants (hardcoded per the harness contract).
B, CIN, P, H, W = 64, 2048, 512, 14, 14
EPS = 1e-5
N = H * W            # 196 pixels
NCORES = 8
BPC = B // NCORES    # 8 images per core
NPAIR = BPC // 2     # 4 image pairs per core
NPAD = 256           # padded free dim for fp32r full-rate matmuls
KC1 = CIN // 128     # 16 input-channel chunks for conv1 / output chunks conv3
PC = P // 128        # 4 chunks of the 512-dim
N2 = 2 * N           # 392 = free dim for image-pair matmuls

# n/m chunking of the 196-pixel dim: 128 + 68
NCHUNKS = [(0, 128), (128, 68)]

_CACHE = {}


def _build(repeat=1):
    import concourse.bass as bass  # noqa: F401
    import concourse.mybir as mybir
    import concourse.tile as tile
    from concourse import bacc
    from concourse.masks import make_identity

    f32 = mybir.dt.float32
    f32r = mybir.dt.float32r

    nc = bacc.Bacc(None, target_bir_lowering=False, debug=False)

    # DRAM parameters. Matmul operands are declared float32r (same 32-bit
    # storage; the PE rounds internally) so the DMA'd tiles are legal fp32r
    # matmul inputs.
    x_d = nc.declare_dram_parameter("x", [KC1, 128, BPC * N], f32r, isOutput=False)
    w1t_d = nc.declare_dram_parameter("w1t", [KC1, 128, P], f32r, isOutput=False)
    wqkt_d = nc.declare_dram_parameter("wqkt", [PC, 128, 2 * P], f32r, isOutput=False)
    wvt_d = nc.declare_dram_parameter("wvt", [PC, 128, P], f32r, isOutput=False)
    w3t_d = nc.declare_dram_parameter("w3t", [PC, 128, CIN], f32r, isOutput=False)
    pos_d = nc.declare_dram_parameter("pos", [PC, 128, N], f32r, isOutput=False)
    t1_d = nc.declare_dram_parameter("t1", [128, PC], f32, isOutput=False)
    s2_d = nc.declare_dram_parameter("s2", [128, PC], f32, isOutput=False)
    t2_d = nc.declare_dram_parameter("t2", [128, PC], f32, isOutput=False)
    t3_d = nc.declare_dram_parameter("t3", [128, KC1], f32, isOutput=False)
    y_d = nc.declare_dram_parameter("y", [KC1, 128, BPC * N], f32, isOutput=True)

    with tile.TileContext(nc) as tc:
        with (
            tc.tile_pool(name="const", bufs=1) as const,
            tc.tile_pool(name="xp", bufs=2) as xp,
            tc.tile_pool(name="h1p", bufs=1) as h1p,
            tc.tile_pool(name="qkp", bufs=1) as qkp,
            tc.tile_pool(name="h2p", bufs=1) as h2p,
            tc.tile_pool(name="attp", bufs=2) as attp,
            tc.tile_pool(name="outp", bufs=3) as outp,
            tc.tile_pool(name="ps_mm", bufs=3, space="PSUM") as ps_mm,
            tc.tile_pool(name="ps_sm", bufs=4, space="PSUM") as ps_sm,
            tc.tile_pool(name="ps_tr", bufs=1, space="PSUM") as ps_tr,
        ):
            # ---- constants / weights (loaded once) ----
            w1t = const.tile([128, KC1, P], f32r)
            nc.sync.dma_start(out=w1t, in_=x_dma_rearr(w1t_d))
            wqkt = const.tile([128, PC, 2 * P], f32r)
            nc.sync.dma_start(out=wqkt, in_=x_dma_rearr(wqkt_d))
            wvt = const.tile([128, PC, P], f32r)
            nc.sync.dma_start(out=wvt, in_=x_dma_rearr(wvt_d))
            w3t = const.tile([128, PC, CIN], f32r)
            nc.sync.dma_start(out=w3t, in_=x_dma_rearr(w3t_d))
            pos = const.tile([128, PC, N], f32r)
            nc.sync.dma_start(out=pos, in_=x_dma_rearr(pos_d))
            t1 = const.tile([128, PC], f32)
            nc.sync.dma_start(out=t1, in_=t1_d[:, :])
            s2 = const.tile([128, PC], f32)
            nc.sync.dma_start(out=s2, in_=s2_d[:, :])
            t2 = const.tile([128, PC], f32)
            nc.sync.dma_start(out=t2, in_=t2_d[:, :])
            t3 = const.tile([128, KC1], f32)
            nc.sync.dma_start(out=t3, in_=t3_d[:, :])
            ident = const.tile([128, 128], f32)
            make_identity(nc, ident)

            Exp = mybir.ActivationFunctionType.Exp
            Relu = mybir.ActivationFunctionType.Relu
            Copy = mybir.ActivationFunctionType.Copy

            import contextlib
            loop_cm = (tc.For_i(0, repeat, 1) if repeat > 1
                       else contextlib.nullcontext())
            with loop_cm:
              for pair in range(NPAIR):
                nsl = slice(pair * N2, (pair + 1) * N2)

                # ---- load x for this pair: [128, 16, 2*196] fp32(r) ----
                x_t = xp.tile([128, KC1, N2], f32r, name=f"x_{pair}", tag="x")
                for kq in range(4):
                    nc.sync.dma_start(
                        out=x_t[:, kq * 4:(kq + 1) * 4, :],
                        in_=x_d[kq * 4:(kq + 1) * 4, :, nsl].rearrange(
                            "k p n -> p k n"),
                    )
                x_f = x_t.bitcast(f32)

                # ---- conv1 + bn1 + relu -> h1 [128, 4, 392] ----
                h1 = h1p.tile([128, PC, N2], f32r, name=f"h1_{pair}", tag="h1")
                for oc in range(PC):
                    cps = ps_mm.tile([128, 512], f32, name="cps", tag="mm")
                    for kc in range(KC1):
                        nc.tensor.matmul(
                            cps[:, :N2],
                            w1t[:, kc, oc * 128:(oc + 1) * 128],
                            x_t[:, kc, :],
                            start=(kc == 0), stop=(kc == KC1 - 1),
                        )
                    nc.scalar.activation(h1[:, oc, :], cps[:, :N2], Relu,
                                         bias=t1[:, oc:oc + 1])

                # ---- q/k projection -> q_sb/k_sb [128, 4, 2, 256] (padded) ----
                q_sb = qkp.tile([128, PC, 2, NPAD], f32r, name=f"q_{pair}", tag="q")
                k_sb = qkp.tile([128, PC, 2, NPAD], f32r, name=f"k_{pair}", tag="k")
                nc.vector.memset(q_sb.bitcast(f32)[:, :, :, N:], 0.0)
                nc.vector.memset(k_sb.bitcast(f32)[:, :, :, N:], 0.0)
                for oc in range(2 * PC):
                    qps = ps_mm.tile([128, 512], f32, name="qps", tag="mm")
                    for pc in range(PC):
                        nc.tensor.matmul(
                            qps[:, :N2],
                            wqkt[:, pc, oc * 128:(oc + 1) * 128],
                            h1[:, pc, :],
                            start=(pc == 0), stop=(pc == PC - 1),
                        )
                    dst = q_sb if oc < PC else k_sb
                    c4 = oc % PC
                    for j in range(2):
                        nc.vector.tensor_copy(
                            dst[:, c4, j, :N], qps[:, j * N:(j + 1) * N])

                # ---- per-image attention ----
                vT_list = []
                attnT_list = []
                for j in range(2):
                    # v^T directly: vT[m, c] = sum_p h1[p, m] wvt[p, c]
                    vT = attp.tile([128, 2, P], f32r, name=f"vT_{pair}_{j}",
                                   tag="vT")
                    for mi, (m0, msz) in enumerate(NCHUNKS):
                        vps = ps_mm.tile([128, 512], f32, name="vps", tag="mm")
                        for pc in range(PC):
                            nc.tensor.matmul(
                                vps[:msz, :],
                                h1[:, pc, j * N + m0:j * N + m0 + msz],
                                wvt[:, pc, :],
                                start=(pc == 0), stop=(pc == PC - 1),
                            )
                        nc.vector.tensor_copy(vT[:msz, mi, :], vps[:msz, :])

                    # attn^T [128, 2, 256] fp32r (padded cols zeroed)
                    attnT = attp.tile([128, 2, NPAD], f32r,
                                      name=f"aT_{pair}_{j}", tag="attnT")
                    nc.vector.memset(attnT.bitcast(f32)[:, :, N:], 0.0)

                    for ni, (n0, nsz) in enumerate(NCHUNKS):
                        lps = ps_sm.tile([128, NPAD], f32, name="lps",
                                         tag="small")
                        # cc: sum_c q[c, n-slice]^T k[c, :]
                        for pc in range(PC):
                            nc.tensor.matmul(
                                lps[:nsz, :],
                                q_sb[:, pc, j, n0:n0 + nsz],
                                k_sb[:, pc, j, :],
                                start=(pc == 0), stop=False,
                            )
                        # cp: sum_c pos[c, n-slice]^T q[c, :]
                        for pc in range(PC):
                            nc.tensor.matmul(
                                lps[:nsz, :],
                                pos[:, pc, n0:n0 + nsz],
                                q_sb[:, pc, j, :],
                                start=False, stop=(pc == PC - 1),
                            )
                        # softmax over free dim (no max-subtraction needed;
                        # logits are O(40) max, exp stays finite in fp32)
                        p_raw = attp.tile([128, N], f32, name="p_raw",
                                          tag="p_raw")
                        ssum = attp.tile([128, 1], f32, name="ssum", tag="ss")
                        nc.scalar.activation(p_raw[:nsz, :], lps[:nsz, :N],
                                             Exp, accum_out=ssum[:nsz, :])
                        rsum = attp.tile([128, 1], f32, name="rsum", tag="rs")
                        nc.vector.reciprocal(rsum[:nsz, :], ssum[:nsz, :])
                        p_nrm = attp.tile([128, N], f32, name="p_nrm",
                                          tag="p_nrm")
                        nc.vector.tensor_scalar_mul(p_nrm[:nsz, :],
                                                    p_raw[:nsz, :],
                                                    rsum[:nsz, :])
                        # transpose normalized attn into attnT[m, n-slice]
                        for mi, (m0, msz) in enumerate(NCHUNKS):
                            tps = ps_tr.tile([128, 128], f32, name="tps",
                                             tag="tr")
                            nc.tensor.transpose(tps[:msz, :nsz],
                                                p_nrm[:nsz, m0:m0 + msz],
                                                ident[:nsz, :nsz])
                            nc.scalar.activation(attnT[:msz, mi, n0:n0 + nsz],
                                                 tps[:msz, :nsz], Copy)
                    vT_list.append(vT)
                    attnT_list.append(attnT)

                # ---- attention output + bn2 + relu -> h2 [128, 4, 2, 196] ----
                h2 = h2p.tile([128, PC, 2, N], f32r, name=f"h2_{pair}", tag="h2")
                for j in range(2):
                    vT = vT_list[j]
                    attnT = attnT_list[j]
                    for c4 in range(PC):
                        aps = ps_sm.tile([128, NPAD], f32, name="aps",
                                         tag="small")
                        for mi, (m0, msz) in enumerate(NCHUNKS):
                            nc.tensor.matmul(
                                aps[:, :],
                                vT[:msz, mi, c4 * 128:(c4 + 1) * 128],
                                attnT[:msz, mi, :],
                                start=(mi == 0), stop=(mi == 1),
                            )
                        nc.scalar.activation(h2[:, c4, j, :], aps[:, :N],
                                             Relu, bias=t2[:, c4:c4 + 1],
                                             scale=s2[:, c4:c4 + 1])

                # ---- conv3 + bn3 + residual + relu -> y ----
                for oc in range(KC1):
                    ops = ps_mm.tile([128, 512], f32, name="ops", tag="mm")
                    for pc in range(PC):
                        nc.tensor.matmul(
                            ops[:, :N2],
                            w3t[:, pc, oc * 128:(oc + 1) * 128],
                            h2[:, pc, :, :],
                            start=(pc == 0), stop=(pc == PC - 1),
                        )
                    tmp = outp.tile([128, N2], f32, name="tmp", tag="tmp")
                    # tmp = (conv3 + t3) + x
                    nc.vector.scalar_tensor_tensor(
                        tmp, ops[:, :N2], t3[:, oc:oc + 1], x_f[:, oc, :],
                        op0=mybir.AluOpType.add, op1=mybir.AluOpType.add)
                    y_sb = outp.tile([128, N2], f32, name="y_sb", tag="y_sb")
                    nc.scalar.activation(y_sb, tmp, Relu)
                    nc.sync.dma_start(out=y_d[oc, :, nsl], in_=y_sb)

    nc.compile()
    return nc


def x_dma_rearr(d):
    return d[:, :, :].rearrange("k p o -> p k o")


def _prep_inputs(x, w1, g1, b1, m1, v1, wqkv, rel_h, rel_w,
                 g2, b2, m2, v2, w3, g3, b3, m3, v3):
    f = np.float32
    x = np.ascontiguousarray(x, f)
    s1 = (g1 / np.sqrt(v1 + EPS)).astype(f)
    t1 = (b1 - m1 * s1).astype(f)
    s2 = (g2 / np.sqrt(v2 + EPS)).astype(f)
    t2 = (b2 - m2 * s2).astype(f)
    s3 = (g3 / np.sqrt(v3 + EPS)).astype(f)
    t3 = (b3 - m3 * s3).astype(f)

    w1p = (w1 * s1[:, None]).astype(f)                    # [512, 2048]
    w1t = np.ascontiguousarray(w1p.T).reshape(KC1, 128, P)
    wqk = wqkv[:2 * P].astype(f)                          # [1024, 512]
    wqkt = np.ascontiguousarray(wqk.T).reshape(PC, 128, 2 * P)
    wv = wqkv[2 * P:].astype(f)                           # [512, 512]
    wvt = np.ascontiguousarray(wv.T).reshape(PC, 128, P)
    w3p = (w3 * s3[:, None]).astype(f)                    # [2048, 512]
    w3t = np.ascontiguousarray(w3p.T).reshape(PC, 128, CIN)
    pos = (rel_h + rel_w).reshape(P, N).astype(f).reshape(PC, 128, N)

    t1_h = np.ascontiguousarray(t1.reshape(PC, 128).T)
    s2_h = np.ascontiguousarray(s2.reshape(PC, 128).T)
    t2_h = np.ascontiguousarray(t2.reshape(PC, 128).T)
    t3_h = np.ascontiguousarray(t3.reshape(KC1, 128).T)

    shared = dict(w1t=w1t, wqkt=wqkt, wvt=wvt, w3t=w3t, pos=pos,
                  t1=t1_h, s2=s2_h, t2=t2_h, t3=t3_h)

    in_maps = []
    for c in range(NCORES):
        xc = x[c * BPC:(c + 1) * BPC].reshape(BPC, KC1, 128, N)
        xc = np.ascontiguousarray(xc.transpose(1, 2, 0, 3)).reshape(
            KC1, 128, BPC * N)
        in_maps.append(dict(shared, x=xc))
    return in_maps


def _run(in_maps, trace=False):
    from concourse.bass_utils import run_bass_kernel_spmd
    if "nc" not in _CACHE:
        _CACHE["nc"] = _build()
    nc = _CACHE["nc"]
    return run_bass_kernel_spmd(nc, in_maps, core_ids=list(range(NCORES)),
                                trace=trace)


def _assemble(results):
    out = np.empty((B, CIN, H, W), np.float32)
    for c in range(NCORES):
        yc = results[c]["y"].reshape(KC1, 128, BPC, N)
        out[c * BPC:(c + 1) * BPC] = yc.transpose(2, 0, 1, 3).reshape(
            BPC, CIN, H, W)
    return out


def kernel(**inputs):
    in_maps = _prep_inputs(**inputs)
    res = _run(in_maps)
    return _assemble(res.results)



# revision 3
# speedup vs baseline: 1.2872x; 1.2872x over previous
"""Trainium2 Bass kernel for the MHSA bottleneck block.

Contract: kernel(**inputs) takes the FULL unsharded inputs (as produced by
setup_inputs()) and returns the FULL [64, 2048, 14, 14] float32 output.
Internally shards data-parallel over batch: 8 images per NeuronCore, 8 cores.

All matmul operands are bf16 (error budget is 2e-2; bf16 lands ~1e-3).
PSUM accumulation stays fp32. DMA issue order is arranged so the first
conv1 matmul only waits on x(pair0) + w1t instead of the whole weight set.
"""
import sys

sys.path.insert(0, '/opt/trn_rl_repo')

import numpy as np
import ml_dtypes

BF16 = ml_dtypes.bfloat16

# Problem constants (hardcoded per the harness contract).
B, CIN, P, H, W = 64, 2048, 512, 14, 14
EPS = 1e-5
N = H * W            # 196 pixels
NCORES = 8
BPC = B // NCORES    # 8 images per core
NPAIR = BPC // 2     # 4 image pairs per core
KC1 = CIN // 128     # 16 input-channel chunks for conv1 / output chunks conv3
PC = P // 128        # 4 chunks of the 512-dim
N2 = 2 * N           # 392 = free dim for image-pair matmuls

# n/m chunking of the 196-pixel dim: 128 + 68
NCHUNKS = [(0, 128), (128, 68)]

_CACHE = {}


def _build():
    import concourse.bass as bass  # noqa: F401
    import concourse.mybir as mybir
    import concourse.tile as tile
    from concourse import bacc
    from concourse.masks import make_identity

    f32 = mybir.dt.float32
    bf16 = mybir.dt.bfloat16

    nc = bacc.Bacc(None, target_bir_lowering=False, debug=False)

    # DRAM parameters, partition-major so each DMA is one long line per
    # partition.
    x_d = nc.declare_dram_parameter("x", [128, NPAIR, KC1 * N2], bf16,
                                    isOutput=False)
    w1t_d = nc.declare_dram_parameter("w1t", [128, KC1, P], bf16,
                                      isOutput=False)
    wqkt_d = nc.declare_dram_parameter("wqkt", [128, PC, 2 * P], bf16,
                                       isOutput=False)
    wvt_d = nc.declare_dram_parameter("wvt", [128, PC, P], bf16,
                                      isOutput=False)
    w3t_d = nc.declare_dram_parameter("w3t", [128, PC, CIN], bf16,
                                      isOutput=False)
    pos_d = nc.declare_dram_parameter("pos", [128, PC, N], bf16,
                                      isOutput=False)
    t1_d = nc.declare_dram_parameter("t1", [128, PC], f32, isOutput=False)
    s2_d = nc.declare_dram_parameter("s2", [128, PC], f32, isOutput=False)
    t2_d = nc.declare_dram_parameter("t2", [128, PC], f32, isOutput=False)
    t3_d = nc.declare_dram_parameter("t3", [128, KC1], f32, isOutput=False)
    y_d = nc.declare_dram_parameter("y", [128, NPAIR, KC1 * N2], bf16,
                                    isOutput=True)

    with tile.TileContext(nc) as tc:
        with (
            tc.tile_pool(name="const", bufs=1) as const,
            tc.tile_pool(name="xp", bufs=3) as xp,
            tc.tile_pool(name="h1p", bufs=2) as h1p,
            tc.tile_pool(name="qkp", bufs=2) as qkp,
            tc.tile_pool(name="h2p", bufs=2) as h2p,
            tc.tile_pool(name="attp", bufs=2) as attp,
            tc.tile_pool(name="outp", bufs=2) as outp,
            tc.tile_pool(name="ps_mm", bufs=3, space="PSUM") as ps_mm,
            tc.tile_pool(name="ps_sm", bufs=4, space="PSUM") as ps_sm,
            tc.tile_pool(name="ps_tr", bufs=1, space="PSUM") as ps_tr,
        ):
            # ---- pair-0 x first, then weights in order of first use ----
            x_tiles = []
            for pair in range(NPAIR):
                if pair < 2:
                    x_t = xp.tile([128, KC1, N2], bf16, name=f"x_{pair}",
                                  tag="x")
                    nc.sync.dma_start(
                        out=x_t, in_=x_d[:, pair, :].rearrange(
                            "p (k n) -> p k n", k=KC1))
                    x_tiles.append(x_t)
                if pair == 0:
                    w1t = const.tile([128, KC1, P], bf16)
                    nc.sync.dma_start(out=w1t, in_=w1t_d[:, :, :])
                    t1 = const.tile([128, PC], f32)
                    nc.sync.dma_start(out=t1, in_=t1_d[:, :])
                    wqkt = const.tile([128, PC, 2 * P], bf16)
                    nc.sync.dma_start(out=wqkt, in_=wqkt_d[:, :, :])
                    wvt = const.tile([128, PC, P], bf16)
                    nc.sync.dma_start(out=wvt, in_=wvt_d[:, :, :])
                    pos = const.tile([128, PC, N], bf16)
                    nc.sync.dma_start(out=pos, in_=pos_d[:, :, :])
                    s2 = const.tile([128, PC], f32)
                    nc.sync.dma_start(out=s2, in_=s2_d[:, :])
                    t2 = const.tile([128, PC], f32)
                    nc.sync.dma_start(out=t2, in_=t2_d[:, :])
                if pair == 1:
                    w3t = const.tile([128, PC, CIN], bf16)
                    nc.sync.dma_start(out=w3t, in_=w3t_d[:, :, :])
                    t3 = const.tile([128, KC1], f32)
                    nc.sync.dma_start(out=t3, in_=t3_d[:, :])

            ident = const.tile([128, 128], bf16)
            make_identity(nc, ident)

            Exp = mybir.ActivationFunctionType.Exp
            Relu = mybir.ActivationFunctionType.Relu
            Copy = mybir.ActivationFunctionType.Copy

            for pair in range(NPAIR):
                if pair < 2:
                    x_t = x_tiles[pair]
                else:
                    x_t = xp.tile([128, KC1, N2], bf16, name=f"x_{pair}",
                                  tag="x")
                    nc.sync.dma_start(
                        out=x_t, in_=x_d[:, pair, :].rearrange(
                            "p (k n) -> p k n", k=KC1))

                # ---- conv1 + bn1 + relu -> h1 [128, 4, 392] bf16 ----
                h1 = h1p.tile([128, PC, N2], bf16, name=f"h1_{pair}",
                              tag="h1")
                for oc in range(PC):
                    cps = ps_mm.tile([128, 512], f32, name="cps", tag="mm")
                    for kc in range(KC1):
                        nc.tensor.matmul(
                            cps[:, :N2],
                            w1t[:, kc, oc * 128:(oc + 1) * 128],
                            x_t[:, kc, :],
                            start=(kc == 0), stop=(kc == KC1 - 1),
                        )
                    nc.scalar.activation(h1[:, oc, :], cps[:, :N2], Relu,
                                         bias=t1[:, oc:oc + 1])

                # ---- q/k projection -> qk_sb [128, 8, 392] bf16 ----
                # oc 0-3 = q chunks, oc 4-7 = k chunks; images stay
                # interleaved in the 392 free dim (j*196 offsets).
                qk_sb = qkp.tile([128, 2 * PC, N2], bf16, name=f"qk_{pair}",
                                 tag="qk")
                for oc in range(2 * PC):
                    qps = ps_mm.tile([128, 512], f32, name="qps", tag="mm")
                    for pc in range(PC):
                        nc.tensor.matmul(
                            qps[:, :N2],
                            wqkt[:, pc, oc * 128:(oc + 1) * 128],
                            h1[:, pc, :],
                            start=(pc == 0), stop=(pc == PC - 1),
                        )
                    nc.vector.tensor_copy(qk_sb[:, oc, :], qps[:, :N2])

                # ---- per-image attention ----
                vT_list = []
                attnT_list = []
                for j in range(2):
                    # v^T directly: vT[m, c] = sum_p h1[p, m] wvt[p, c]
                    vT = attp.tile([128, 2, P], bf16, name=f"vT_{pair}_{j}",
                                   tag="vT")
                    for mi, (m0, msz) in enumerate(NCHUNKS):
                        vps = ps_mm.tile([128, 512], f32, name="vps",
                                         tag="mm")
                        for pc in range(PC):
                            nc.tensor.matmul(
                                vps[:msz, :],
                                h1[:, pc, j * N + m0:j * N + m0 + msz],
                                wvt[:, pc, :],
                                start=(pc == 0), stop=(pc == PC - 1),
                            )
                        nc.vector.tensor_copy(vT[:msz, mi, :], vps[:msz, :])

                    # attn^T [128, 2, 196] bf16
                    attnT = attp.tile([128, 2, N], bf16,
                                      name=f"aT_{pair}_{j}", tag="attnT")

                    for ni, (n0, nsz) in enumerate(NCHUNKS):
                        lps = ps_sm.tile([128, N], f32, name="lps",
                                         tag="small")
                        # cc: sum_c q[c, n-slice]^T k[c, :]
                        for pc in range(PC):
                            nc.tensor.matmul(
                                lps[:nsz, :],
                                qk_sb[:, pc, j * N + n0:j * N + n0 + nsz],
                                qk_sb[:, PC + pc, j * N:(j + 1) * N],
                                start=(pc == 0), stop=False,
                            )
                        # cp: sum_c pos[c, n-slice]^T q[c, :]
                        for pc in range(PC):
                            nc.tensor.matmul(
                                lps[:nsz, :],
                                pos[:, pc, n0:n0 + nsz],
                                qk_sb[:, pc, j * N:(j + 1) * N],
                                start=False, stop=(pc == PC - 1),
                            )
                        # softmax over free dim (logits are O(40) max, exp
                        # stays finite in fp32; no max-subtraction needed)
                        p_raw = attp.tile([128, N], f32, name="p_raw",
                                          tag="p_raw")
                        ssum = attp.tile([128, 1], f32, name="ssum",
                                         tag="ss")
                        nc.scalar.activation(p_raw[:nsz, :], lps[:nsz, :],
                                             Exp, accum_out=ssum[:nsz, :])
                        rsum = attp.tile([128, 1], f32, name="rsum",
                                         tag="rs")
                        nc.vector.reciprocal(rsum[:nsz, :], ssum[:nsz, :])
                        p_nrm = attp.tile([128, N], bf16, name="p_nrm",
                                          tag="p_nrm")
                        nc.vector.tensor_scalar_mul(p_nrm[:nsz, :],
                                                    p_raw[:nsz, :],
                                                    rsum[:nsz, :])
                        # transpose normalized attn into attnT[m, n-slice]
                        for mi, (m0, msz) in enumerate(NCHUNKS):
                            tps = ps_tr.tile([128, 128], bf16, name="tps",
                                             tag="tr")
                            nc.tensor.transpose(tps[:msz, :nsz],
                                                p_nrm[:nsz, m0:m0 + msz],
                                                ident[:nsz, :nsz])
                            nc.scalar.activation(attnT[:msz, mi,
                                                       n0:n0 + nsz],
                                                 tps[:msz, :nsz], Copy)
                    vT_list.append(vT)
                    attnT_list.append(attnT)

                # ---- attention output + bn2 + relu -> h2 ----
                h2 = h2p.tile([128, PC, 2, N], bf16, name=f"h2_{pair}",
                              tag="h2")
                for j in range(2):
                    vT = vT_list[j]
                    attnT = attnT_list[j]
                    for c4 in range(PC):
                        aps = ps_sm.tile([128, N], f32, name="aps",
                                         tag="small")
                        for mi, (m0, msz) in enumerate(NCHUNKS):
                            nc.tensor.matmul(
                                aps[:, :],
                                vT[:msz, mi, c4 * 128:(c4 + 1) * 128],
                                attnT[:msz, mi, :],
                                start=(mi == 0), stop=(mi == 1),
                            )
                        nc.scalar.activation(h2[:, c4, j, :], aps[:, :],
                                             Relu, bias=t2[:, c4:c4 + 1],
                                             scale=s2[:, c4:c4 + 1])

                # ---- conv3 + bn3 + residual + relu -> y ----
                ysb = outp.tile([128, KC1, N2], bf16, name=f"y_{pair}",
                                tag="ysb")
                for oc in range(KC1):
                    ops = ps_mm.tile([128, 512], f32, name="ops", tag="mm")
                    for pc in range(PC):
                        nc.tensor.matmul(
                            ops[:, :N2],
                            w3t[:, pc, oc * 128:(oc + 1) * 128],
                            h2[:, pc, :, :],
                            start=(pc == 0), stop=(pc == PC - 1),
                        )
                    tmp = outp.tile([128, N2], f32, name="tmp", tag="tmp")
                    # tmp = (conv3 + t3) + x
                    nc.vector.scalar_tensor_tensor(
                        tmp, ops[:, :N2], t3[:, oc:oc + 1], x_t[:, oc, :],
                        op0=mybir.AluOpType.add, op1=mybir.AluOpType.add)
                    nc.scalar.activation(ysb[:, oc, :], tmp, Relu)
                nc.sync.dma_start(
                    out=y_d[:, pair, :].rearrange("p (k n) -> p k n",
                                                  k=KC1),
                    in_=ysb)

    nc.compile()
    return nc


def _prep_inputs(x, w1, g1, b1, m1, v1, wqkv, rel_h, rel_w,
                 g2, b2, m2, v2, w3, g3, b3, m3, v3):
    f = np.float32
    s1 = (g1 / np.sqrt(v1 + EPS)).astype(f)
    t1 = (b1 - m1 * s1).astype(f)
    s2 = (g2 / np.sqrt(v2 + EPS)).astype(f)
    t2 = (b2 - m2 * s2).astype(f)
    s3 = (g3 / np.sqrt(v3 + EPS)).astype(f)
    t3 = (b3 - m3 * s3).astype(f)

    w1p = (w1 * s1[:, None]).astype(f)                    # [512, 2048]
    w1t = np.ascontiguousarray(
        w1p.T.reshape(KC1, 128, P).transpose(1, 0, 2)).astype(BF16)
    wqk = wqkv[:2 * P].astype(f)                          # [1024, 512]
    wqkt = np.ascontiguousarray(
        wqk.T.reshape(PC, 128, 2 * P).transpose(1, 0, 2)).astype(BF16)
    wv = wqkv[2 * P:].astype(f)                           # [512, 512]
    wvt = np.ascontiguousarray(
        wv.T.reshape(PC, 128, P).transpose(1, 0, 2)).astype(BF16)
    w3p = (w3 * s3[:, None]).astype(f)                    # [2048, 512]
    w3t = np.ascontiguousarray(
        w3p.T.reshape(PC, 128, CIN).transpose(1, 0, 2)).astype(BF16)
    pos = (rel_h + rel_w).reshape(P, N).astype(f)
    pos = np.ascontiguousarray(
        pos.reshape(PC, 128, N).transpose(1, 0, 2)).astype(BF16)

    t1_h = np.ascontiguousarray(t1.reshape(PC, 128).T)
    s2_h = np.ascontiguousarray(s2.reshape(PC, 128).T)
    t2_h = np.ascontiguousarray(t2.reshape(PC, 128).T)
    t3_h = np.ascontiguousarray(t3.reshape(KC1, 128).T)

    shared = dict(w1t=w1t, wqkt=wqkt, wvt=wvt, w3t=w3t, pos=pos,
                  t1=t1_h, s2=s2_h, t2=t2_h, t3=t3_h)

    xb = np.asarray(x, f).astype(BF16)
    in_maps = []
    for c in range(NCORES):
        # [BPC, CIN, H, W] -> [128, NPAIR, KC1 * N2] with free layout
        # per pair: [kc, j, n]
        xc = xb[c * BPC:(c + 1) * BPC].reshape(NPAIR, 2, KC1, 128, N)
        xc = np.ascontiguousarray(xc.transpose(3, 0, 2, 1, 4)).reshape(
            128, NPAIR, KC1 * N2)
        in_maps.append(dict(shared, x=xc))
    return in_maps


def _run(in_maps, trace=False):
    from concourse.bass_utils import run_bass_kernel_spmd
    if "nc" not in _CACHE:
        _CACHE["nc"] = _build()
    nc = _CACHE["nc"]
    return run_bass_kernel_spmd(nc, in_maps, core_ids=list(range(NCORES)),
                                trace=trace)


def _assemble(results):
    out = np.empty((B, CIN, H, W), np.float32)
    for c in range(NCORES):
        # [128, NPAIR, KC1*N2] -> [BPC, CIN, H, W]
        yc = results[c]["y"].reshape(128, NPAIR, KC1, 2, N).astype(
            np.float32)
        out[c * BPC:(c + 1) * BPC] = yc.transpose(1, 3, 2, 0, 4).reshape(
            BPC, CIN, H, W)
    return out


def kernel(**inputs):
    in_maps = _prep_inputs(**inputs)
    res = _run(in_maps)
    return _assemble(res.results)


# revision 16
# speedup vs baseline: 1.4563x; 1.1314x over previous
"""Trainium2 Bass kernel for the MHSA bottleneck block.

Contract: kernel(**inputs) takes the FULL unsharded inputs (as produced by
setup_inputs()) and returns the FULL [64, 2048, 14, 14] float32 output.
Internally shards data-parallel over batch: 8 images per NeuronCore, 8 cores.

Precision plan (error budget 2e-2, lands ~3e-3):
  - conv1 / attention matmuls: bf16 operands, fp32 PSUM accumulate.
  - conv3: fp8e4 DoubleRow (2 rows/cycle), h2 + w3 in fp8. conv3's output
    is small relative to the residual x, so fp8 noise there is damped.
Attention-score algebra: cc = q^T k = h1^T (Wq^T Wk) h1, so only one
projection g = (Wq^T Wk)^T h1 is computed on-chip; cp = pos^T q folds to
P2^T h1 with P2 = Wq^T pos precomputed on host. This removes the q/k
projections entirely.
DMA issue order is arranged so the first conv1 matmul only waits on the
first quarter of x(pair0) + w1t.
"""
import sys

sys.path.insert(0, '/opt/trn_rl_repo')

import numpy as np
import ml_dtypes

BF16 = ml_dtypes.bfloat16
F8E4 = ml_dtypes.float8_e4m3

# Problem constants (hardcoded per the harness contract).
B, CIN, P, H, W = 64, 2048, 512, 14, 14
EPS = 1e-5
N = H * W            # 196 pixels
NCORES = 8
BPC = B // NCORES    # 8 images per core
NPAIR = BPC // 2     # 4 image pairs per core
KC1 = CIN // 128     # 16 input-channel chunks for conv1 / output chunks conv3
PC = P // 128        # 4 chunks of the 512-dim
N2 = 2 * N           # 392 = free dim for image-pair matmuls
N2P = 400            # N2 padded so fp8 DoubleRow strides are 16B-aligned

# n/m chunking of the 196-pixel dim: 128 + 68
NCHUNKS = [(0, 128), (128, 68)]

_CACHE = {}


def _build():
    import concourse.bass as bass  # noqa: F401
    import concourse.mybir as mybir
    import concourse.tile as tile
    from concourse import bacc
    from concourse.masks import make_identity

    f32 = mybir.dt.float32
    bf16 = mybir.dt.bfloat16
    f8 = mybir.dt.float8e4
    DR = mybir.MatmulPerfMode.DoubleRow

    nc = bacc.Bacc(None, target_bir_lowering=False, debug=False)

    # DRAM parameters, partition-major so each DMA is one long line per
    # partition.
    x_d = nc.declare_dram_parameter("x", [128, NPAIR, KC1 * N2], bf16,
                                    isOutput=False)
    w1t_d = nc.declare_dram_parameter("w1t", [128, KC1, P], bf16,
                                      isOutput=False)
    mt_d = nc.declare_dram_parameter("mt", [128, PC, P], bf16,
                                     isOutput=False)
    wvt_d = nc.declare_dram_parameter("wvt", [128, PC, P], bf16,
                                      isOutput=False)
    w3t_d = nc.declare_dram_parameter("w3t", [128, 2 * 2 * CIN], f8,
                                      isOutput=False)
    pos_d = nc.declare_dram_parameter("pos", [128, PC, N], bf16,
                                      isOutput=False)
    t1_d = nc.declare_dram_parameter("t1", [128, PC], f32, isOutput=False)
    s2_d = nc.declare_dram_parameter("s2", [128, PC], f32, isOutput=False)
    t2_d = nc.declare_dram_parameter("t2", [128, PC], f32, isOutput=False)
    t3_d = nc.declare_dram_parameter("t3", [128, KC1], f32, isOutput=False)
    y_d = nc.declare_dram_parameter("y", [128, NPAIR, KC1 * N2], bf16,
                                    isOutput=True)

    XCH = 4              # x/w1 startup chunks for pair 0
    KCC = KC1 // XCH     # 4 kc per chunk

    with tile.TileContext(nc) as tc:
        with (
            tc.tile_pool(name="const", bufs=1) as const,
            tc.tile_pool(name="xp", bufs=3) as xp,
            tc.tile_pool(name="x0p", bufs=1) as x0p,
            tc.tile_pool(name="h1p", bufs=2) as h1p,
            tc.tile_pool(name="qkp", bufs=2) as qkp,
            tc.tile_pool(name="h2p", bufs=2) as h2p,
            tc.tile_pool(name="attp", bufs=2) as attp,
            tc.tile_pool(name="outp", bufs=2) as outp,
            tc.tile_pool(name="ps_mm", bufs=3, space="PSUM") as ps_mm,
            tc.tile_pool(name="ps_sm", bufs=4, space="PSUM") as ps_sm,
            tc.tile_pool(name="ps_tr", bufs=1, space="PSUM") as ps_tr,
        ):
            # ---- startup: interleave x(pair0) and w1t quarter-chunks so the
            # first conv1 matmul waits on only 1/4 of each ----
            x0c = []
            w1c = []
            for c in range(XCH):
                xt = x0p.tile([128, KCC, N2], bf16, name=f"x0_{c}")
                nc.sync.dma_start(
                    out=xt,
                    in_=x_d[:, 0, c * KCC * N2:(c + 1) * KCC * N2].rearrange(
                        "p (k n) -> p k n", k=KCC))
                x0c.append(xt)
                wt = const.tile([128, KCC, P], bf16, name=f"w1_{c}")
                nc.sync.dma_start(out=wt, in_=w1t_d[:, c * KCC:(c + 1) * KCC, :])
                w1c.append(wt)
            t1 = const.tile([128, PC], f32)
            nc.sync.dma_start(out=t1, in_=t1_d[:, :])
            mt = const.tile([128, PC, P], bf16)
            nc.sync.dma_start(out=mt, in_=mt_d[:, :, :])
            wvt = const.tile([128, PC, P], bf16)
            nc.sync.dma_start(out=wvt, in_=wvt_d[:, :, :])
            pos = const.tile([128, PC, N], bf16)
            nc.sync.dma_start(out=pos, in_=pos_d[:, :, :])
            s2 = const.tile([128, PC], f32)
            nc.sync.dma_start(out=s2, in_=s2_d[:, :])
            t2 = const.tile([128, PC], f32)
            nc.sync.dma_start(out=t2, in_=t2_d[:, :])
            # x(pair1) ahead of w3 so pair1's conv1 isn't DMA-gated
            x1_t = xp.tile([128, KC1, N2], bf16, name="x_1", tag="x")
            nc.sync.dma_start(
                out=x1_t,
                in_=x_d[:, 1, :].rearrange("p (k n) -> p k n", k=KC1))
            w3t = const.tile([128, 2, 2, CIN], f8)
            nc.sync.dma_start(
                out=w3t,
                in_=w3t_d[:, :].rearrange("p (a b c) -> p a b c", a=2, b=2))
            t3 = const.tile([128, KC1], f32)
            nc.sync.dma_start(out=t3, in_=t3_d[:, :])

            ident = const.tile([128, 128], bf16)
            make_identity(nc, ident)

            Exp = mybir.ActivationFunctionType.Exp
            Relu = mybir.ActivationFunctionType.Relu
            Copy = mybir.ActivationFunctionType.Copy
            Add = mybir.AluOpType.add
            Max = mybir.AluOpType.max

            for pair in range(NPAIR):
                if pair == 0:
                    xparts = [(x0c[kc // KCC], kc % KCC) for kc in range(KC1)]
                elif pair == 1:
                    xparts = [(x1_t, kc) for kc in range(KC1)]
                else:
                    x_t = xp.tile([128, KC1, N2], bf16, name=f"x_{pair}",
                                  tag="x")
                    nc.sync.dma_start(
                        out=x_t,
                        in_=x_d[:, pair, :].rearrange("p (k n) -> p k n",
                                                      k=KC1))
                    xparts = [(x_t, kc) for kc in range(KC1)]

                # ---- conv1 + bn1 + relu -> h1 [128, 4, 392] bf16 ----
                h1 = h1p.tile([128, PC, N2], bf16, name=f"h1_{pair}",
                              tag="h1")
                for oc in range(PC):
                    cps = ps_mm.tile([128, 512], f32, name="cps", tag="mm")
                    for kc in range(KC1):
                        if pair == 0:
                            wtile, wi = w1c[kc // KCC], kc % KCC
                        else:
                            wtile, wi = w1c[kc // KCC], kc % KCC
                        xtile, xi = xparts[kc]
                        nc.tensor.matmul(
                            cps[:, :N2],
                            wtile[:, wi, oc * 128:(oc + 1) * 128],
                            xtile[:, xi, :],
                            start=(kc == 0), stop=(kc == KC1 - 1),
                        )
                    nc.scalar.activation(h1[:, oc, :], cps[:, :N2], Relu,
                                         bias=t1[:, oc:oc + 1])

                # ---- score projection g = (Wq^T Wk)^T h1 [128, 4, 392] ----
                g_sb = qkp.tile([128, PC, N2], bf16, name=f"g_{pair}",
                                tag="g")
                for oc in range(PC):
                    qps = ps_mm.tile([128, 512], f32, name="qps", tag="mm")
                    for pc in range(PC):
                        nc.tensor.matmul(
                            qps[:, :N2],
                            mt[:, pc, oc * 128:(oc + 1) * 128],
                            h1[:, pc, :],
                            start=(pc == 0), stop=(pc == PC - 1),
                        )
                    nc.vector.tensor_copy(g_sb[:, oc, :], qps[:, :N2])

                # ---- per-image attention ----
                vT_list = []
                attnT_list = []
                for j in range(2):
                    # v^T directly: vT[m, c] = sum_p h1[p, m] wvt[p, c]
                    vT = attp.tile([128, 2, P], bf16, name=f"vT_{pair}_{j}",
                                   tag="vT")
                    for mi, (m0, msz) in enumerate(NCHUNKS):
                        vps = ps_mm.tile([128, 512], f32, name="vps",
                                         tag="mm")
                        for pc in range(PC):
                            nc.tensor.matmul(
                                vps[:msz, :],
                                h1[:, pc, j * N + m0:j * N + m0 + msz],
                                wvt[:, pc, :],
                                start=(pc == 0), stop=(pc == PC - 1),
                            )
                        nc.vector.tensor_copy(vT[:msz, mi, :], vps[:msz, :])

                    # attn^T [128, 2, 196] bf16
                    attnT = attp.tile([128, 2, N], bf16,
                                      name=f"aT_{pair}_{j}", tag="attnT")

                    for ni, (n0, nsz) in enumerate(NCHUNKS):
                        lps = ps_sm.tile([128, N], f32, name="lps",
                                         tag="small")
                        # cc: sum_d h1[d, n-slice]^T g[d, :]
                        for pc in range(PC):
                            nc.tensor.matmul(
                                lps[:nsz, :],
                                h1[:, pc, j * N + n0:j * N + n0 + nsz],
                                g_sb[:, pc, j * N:(j + 1) * N],
                                start=(pc == 0), stop=False,
                            )
                        # cp: sum_d P2[d, n-slice]^T h1[d, :]
                        for pc in range(PC):
                            nc.tensor.matmul(
                                lps[:nsz, :],
                                pos[:, pc, n0:n0 + nsz],
                                h1[:, pc, j * N:(j + 1) * N],
                                start=False, stop=(pc == PC - 1),
                            )
                        # softmax over free dim (logits are O(40) max, exp
                        # stays finite in fp32; no max-subtraction needed)
                        p_raw = attp.tile([128, N], f32, name="p_raw",
                                          tag="p_raw")
                        ssum = attp.tile([128, 1], f32, name="ssum",
                                         tag="ss")
                        nc.scalar.activation(p_raw[:nsz, :], lps[:nsz, :],
                                             Exp, accum_out=ssum[:nsz, :])
                        rsum = attp.tile([128, 1], f32, name="rsum",
                                         tag="rs")
                        nc.vector.reciprocal(rsum[:nsz, :], ssum[:nsz, :])
                        p_nrm = attp.tile([128, N], bf16, name="p_nrm",
                                          tag="p_nrm")
                        nc.vector.tensor_scalar_mul(p_nrm[:nsz, :],
                                                    p_raw[:nsz, :],
                                                    rsum[:nsz, :])
                        # transpose normalized attn into attnT[m, n-slice]
                        for mi, (m0, msz) in enumerate(NCHUNKS):
                            tps = ps_tr.tile([128, 128], bf16, name="tps",
                                             tag="tr")
                            nc.tensor.transpose(tps[:msz, :nsz],
                                                p_nrm[:nsz, m0:m0 + msz],
                                                ident[:nsz, :nsz])
                            nc.scalar.activation(
                                attnT[:msz, mi, n0:n0 + nsz],
                                tps[:msz, :nsz], Copy)
                    vT_list.append(vT)
                    attnT_list.append(attnT)

                # ---- attention output + bn2 + relu -> h2 (fp8, DoubleRow
                # layout [dp, i, j*n] padded to 400) ----
                h2 = h2p.tile([128, 2, 2, N2P], f8, name=f"h2_{pair}",
                              tag="h2")
                nc.vector.memset(h2[:, :, :, N2:], 0.0)
                for j in range(2):
                    vT = vT_list[j]
                    attnT = attnT_list[j]
                    for c4 in range(PC):
                        aps = ps_sm.tile([128, N], f32, name="aps",
                                         tag="small")
                        for mi, (m0, msz) in enumerate(NCHUNKS):
                            nc.tensor.matmul(
                                aps[:, :],
                                vT[:msz, mi, c4 * 128:(c4 + 1) * 128],
                                attnT[:msz, mi, :],
                                start=(mi == 0), stop=(mi == 1),
                            )
                        nc.scalar.activation(
                            h2[:, c4 // 2, c4 % 2, j * N:(j + 1) * N],
                            aps[:, :], Relu, bias=t2[:, c4:c4 + 1],
                            scale=s2[:, c4:c4 + 1])

                # ---- conv3 (fp8 DoubleRow) + bn3 + residual + relu -> y ----
                ysb = outp.tile([128, KC1, N2], bf16, name=f"y_{pair}",
                                tag="ysb")
                for oc in range(KC1):
                    ops = ps_mm.tile([128, 512], f32, name="ops", tag="mm")
                    for dp in range(2):
                        nc.tensor.matmul(
                            ops[:, :N2],
                            w3t[:, dp, :, oc * 128:(oc + 1) * 128],
                            h2[:, dp, :, :N2],
                            start=(dp == 0), stop=(dp == 1),
                            perf_mode=DR,
                        )
                    tmp = outp.tile([128, N2], f32, name="tmp", tag="tmp")
                    xtile, xi = xparts[oc]
                    # tmp = (conv3 + t3) + x
                    nc.vector.scalar_tensor_tensor(
                        tmp, ops[:, :N2], t3[:, oc:oc + 1], xtile[:, xi, :],
                        op0=Add, op1=Add)
                    nc.scalar.activation(ysb[:, oc, :], tmp, Relu)
                    if oc % 4 == 3:
                        g = oc - 3
                        nc.sync.dma_start(
                            out=y_d[:, pair, g * N2:(g + 4) * N2].rearrange(
                                "p (k n) -> p k n", k=4),
                            in_=ysb[:, g:g + 4, :])

    nc.compile()
    return nc


def _prep_inputs(x, w1, g1, b1, m1, v1, wqkv, rel_h, rel_w,
                 g2, b2, m2, v2, w3, g3, b3, m3, v3):
    f = np.float32
    s1 = (g1 / np.sqrt(v1 + EPS)).astype(f)
    t1 = (b1 - m1 * s1).astype(f)
    s2 = (g2 / np.sqrt(v2 + EPS)).astype(f)
    t2 = (b2 - m2 * s2).astype(f)
    s3 = (g3 / np.sqrt(v3 + EPS)).astype(f)
    t3 = (b3 - m3 * s3).astype(f)

    w1p = (w1 * s1[:, None]).astype(f)                    # [512, 2048]
    w1t = np.ascontiguousarray(
        w1p.T.reshape(KC1, 128, P).transpose(1, 0, 2)).astype(BF16)
    # Fold q/k projections: cc = h1^T (Wq^T Wk) h1 -> g = MT^T h1 with
    # MT[e, d] = (Wk^T Wq)[e, d]; cp = pos^T q -> P2^T h1, P2 = Wq^T pos.
    wq = wqkv[:P].astype(np.float64)                      # [512, 512]
    wk = wqkv[P:2 * P].astype(np.float64)
    mtm = wk.T @ wq                                       # [512e, 512d]
    mt = np.ascontiguousarray(
        mtm.reshape(PC, 128, P).transpose(1, 0, 2)).astype(BF16)
    wv = wqkv[2 * P:].astype(f)                           # [512, 512]
    wvt = np.ascontiguousarray(
        wv.T.reshape(PC, 128, P).transpose(1, 0, 2)).astype(BF16)
    w3p = (w3 * s3[:, None]).astype(f)                    # [2048, 512]
    # DoubleRow layout: channel c = dp*256 + i*128 + p
    w3t = np.ascontiguousarray(
        w3p.T.reshape(2, 2, 128, CIN).transpose(2, 0, 1, 3)).reshape(
        128, 2 * 2 * CIN).astype(F8E4)
    posm = (np.asarray(rel_h, np.float64)
            + np.asarray(rel_w, np.float64)).reshape(P, N)
    p2 = np.asarray(wqkv[:P], np.float64).T @ posm        # [512d, 196n]
    pos = np.ascontiguousarray(
        p2.reshape(PC, 128, N).transpose(1, 0, 2)).astype(BF16)

    t1_h = np.ascontiguousarray(t1.reshape(PC, 128).T)
    s2_h = np.ascontiguousarray(s2.reshape(PC, 128).T)
    t2_h = np.ascontiguousarray(t2.reshape(PC, 128).T)
    t3_h = np.ascontiguousarray(t3.reshape(KC1, 128).T)

    shared = dict(w1t=w1t, mt=mt, wvt=wvt, w3t=w3t, pos=pos,
                  t1=t1_h, s2=s2_h, t2=t2_h, t3=t3_h)

    xb = np.asarray(x, f).astype(BF16)
    in_maps = []
    for c in range(NCORES):
        # [BPC, CIN, H, W] -> [128, NPAIR, KC1 * N2] with free layout
        # per pair: [kc, j, n]
        xc = xb[c * BPC:(c + 1) * BPC].reshape(NPAIR, 2, KC1, 128, N)
        xc = np.ascontiguousarray(xc.transpose(3, 0, 2, 1, 4)).reshape(
            128, NPAIR, KC1 * N2)
        in_maps.append(dict(shared, x=xc))
    return in_maps


def _run(in_maps, trace=False):
    from concourse.bass_utils import run_bass_kernel_spmd
    if "nc" not in _CACHE:
        _CACHE["nc"] = _build()
    nc = _CACHE["nc"]
    return run_bass_kernel_spmd(nc, in_maps, core_ids=list(range(NCORES)),
                                trace=trace)


def _assemble(results):
    out = np.empty((B, CIN, H, W), np.float32)
    for c in range(NCORES):
        # [128, NPAIR, KC1*N2] -> [BPC, CIN, H, W]
        yc = results[c]["y"].reshape(128, NPAIR, KC1, 2, N).astype(
            np.float32)
        out[c * BPC:(c + 1) * BPC] = yc.transpose(1, 3, 2, 0, 4).reshape(
            BPC, CIN, H, W)
    return out


def kernel(**inputs):
    in_maps = _prep_inputs(**inputs)
    res = _run(in_maps)
    return _assemble(res.results)


# revision 29
# speedup vs baseline: 1.4643x; 1.0055x over previous
"""Trainium2 Bass kernel for the MHSA bottleneck block.

Contract: kernel(**inputs) takes the FULL unsharded inputs (as produced by
setup_inputs()) and returns the FULL [64, 2048, 14, 14] float32 output.
Internally shards data-parallel over batch: 8 images per NeuronCore, 8 cores.

Precision plan (error budget 2e-2, lands ~3e-3):
  - conv1 / attention matmuls: bf16 operands, fp32 PSUM accumulate.
  - conv3: fp8e4 DoubleRow (2 rows/cycle), h2 + w3 in fp8. conv3's output
    is small relative to the residual x, so fp8 noise there is damped.
Attention-score algebra: cc = q^T k = h1^T (Wq^T Wk) h1, so only one
projection g = (Wq^T Wk)^T h1 is computed on-chip; cp = pos^T q folds to
P2^T h1 with P2 = Wq^T pos precomputed on host. This removes the q/k
projections entirely.
DMA issue order is arranged so the first conv1 matmul only waits on the
first quarter of x(pair0) + w1t.
"""
import sys

sys.path.insert(0, '/opt/trn_rl_repo')

import numpy as np
import ml_dtypes

BF16 = ml_dtypes.bfloat16
F8E4 = ml_dtypes.float8_e4m3

# Problem constants (hardcoded per the harness contract).
B, CIN, P, H, W = 64, 2048, 512, 14, 14
EPS = 1e-5
N = H * W            # 196 pixels
NCORES = 8
BPC = B // NCORES    # 8 images per core
NPAIR = BPC // 2     # 4 image pairs per core
KC1 = CIN // 128     # 16 input-channel chunks for conv1 / output chunks conv3
PC = P // 128        # 4 chunks of the 512-dim
N2 = 2 * N           # 392 = free dim for image-pair matmuls
N2P = 400            # N2 padded so fp8 DoubleRow strides are 16B-aligned

# n/m chunking of the 196-pixel dim: 128 + 68
NCHUNKS = [(0, 128), (128, 68)]

_CACHE = {}


def _build():
    import concourse.bass as bass  # noqa: F401
    import concourse.mybir as mybir
    import concourse.tile as tile
    from concourse import bacc
    from concourse.masks import make_identity

    f32 = mybir.dt.float32
    bf16 = mybir.dt.bfloat16
    f8 = mybir.dt.float8e4
    DR = mybir.MatmulPerfMode.DoubleRow

    nc = bacc.Bacc(None, target_bir_lowering=False, debug=False)

    # DRAM parameters, partition-major so each DMA is one long line per
    # partition.
    x_d = nc.declare_dram_parameter("x", [128, NPAIR, KC1 * N2], bf16,
                                    isOutput=False)
    w1t_d = nc.declare_dram_parameter("w1t", [128, KC1, P], bf16,
                                      isOutput=False)
    mt_d = nc.declare_dram_parameter("mt", [128, 2 * 2 * P], f8,
                                     isOutput=False)
    wvt_d = nc.declare_dram_parameter("wvt", [128, 2 * 2 * P], f8,
                                      isOutput=False)
    w3t_d = nc.declare_dram_parameter("w3t", [128, 2 * 2 * CIN], f8,
                                      isOutput=False)
    pos_d = nc.declare_dram_parameter("pos", [128, PC, N], bf16,
                                      isOutput=False)
    t1_d = nc.declare_dram_parameter("t1", [128, PC], f32, isOutput=False)
    s2_d = nc.declare_dram_parameter("s2", [128, PC], f32, isOutput=False)
    t2_d = nc.declare_dram_parameter("t2", [128, PC], f32, isOutput=False)
    t3_d = nc.declare_dram_parameter("t3", [128, KC1], f32, isOutput=False)
    y_d = nc.declare_dram_parameter("y", [128, NPAIR, KC1 * N2], bf16,
                                    isOutput=True)

    XCH = 4              # x/w1 startup chunks for pair 0
    KCC = KC1 // XCH     # 4 kc per chunk

    with tile.TileContext(nc) as tc:
        with (
            tc.tile_pool(name="const", bufs=1) as const,
            tc.tile_pool(name="xp", bufs=3) as xp,
            tc.tile_pool(name="x0p", bufs=1) as x0p,
            tc.tile_pool(name="h1p", bufs=2) as h1p,
            tc.tile_pool(name="qkp", bufs=2) as qkp,
            tc.tile_pool(name="h2p", bufs=2) as h2p,
            tc.tile_pool(name="attp", bufs=2) as attp,
            tc.tile_pool(name="outp", bufs=2) as outp,
            tc.tile_pool(name="ps_mm", bufs=4, space="PSUM") as ps_mm,
            tc.tile_pool(name="ps_sm", bufs=3, space="PSUM") as ps_sm,
            tc.tile_pool(name="ps_tr", bufs=1, space="PSUM") as ps_tr,
        ):
            # ---- startup: interleave x(pair0) and w1t quarter-chunks so the
            # first conv1 matmul waits on only 1/4 of each ----
            x0c = []
            w1c = []
            for c in range(XCH):
                xt = x0p.tile([128, KCC, N2], bf16, name=f"x0_{c}")
                nc.sync.dma_start(
                    out=xt,
                    in_=x_d[:, 0, c * KCC * N2:(c + 1) * KCC * N2].rearrange(
                        "p (k n) -> p k n", k=KCC))
                x0c.append(xt)
                wt = const.tile([128, KCC, P], bf16, name=f"w1_{c}")
                nc.sync.dma_start(out=wt, in_=w1t_d[:, c * KCC:(c + 1) * KCC, :])
                w1c.append(wt)
            t1 = const.tile([128, PC], f32)
            nc.sync.dma_start(out=t1, in_=t1_d[:, :])
            mt = const.tile([128, 2, 2, P], f8)
            nc.sync.dma_start(
                out=mt,
                in_=mt_d[:, :].rearrange("p (a b c) -> p a b c", a=2, b=2))
            wvt = const.tile([128, 2, 2, P], f8)
            nc.sync.dma_start(
                out=wvt,
                in_=wvt_d[:, :].rearrange("p (a b c) -> p a b c", a=2, b=2))
            pos = const.tile([128, PC, N], bf16)
            nc.sync.dma_start(out=pos, in_=pos_d[:, :, :])
            s2 = const.tile([128, PC], f32)
            nc.sync.dma_start(out=s2, in_=s2_d[:, :])
            t2 = const.tile([128, PC], f32)
            nc.sync.dma_start(out=t2, in_=t2_d[:, :])
            # x(pair1) ahead of w3 so pair1's conv1 isn't DMA-gated
            x1_t = xp.tile([128, KC1, N2], bf16, name="x_1", tag="x")
            nc.sync.dma_start(
                out=x1_t,
                in_=x_d[:, 1, :].rearrange("p (k n) -> p k n", k=KC1))
            w3t = const.tile([128, 2, 2, CIN], f8)
            nc.sync.dma_start(
                out=w3t,
                in_=w3t_d[:, :].rearrange("p (a b c) -> p a b c", a=2, b=2))
            t3 = const.tile([128, KC1], f32)
            nc.sync.dma_start(out=t3, in_=t3_d[:, :])

            ident = const.tile([128, 128], bf16)
            make_identity(nc, ident)

            Exp = mybir.ActivationFunctionType.Exp
            Relu = mybir.ActivationFunctionType.Relu
            Copy = mybir.ActivationFunctionType.Copy
            Add = mybir.AluOpType.add
            Mult = mybir.AluOpType.mult

            for pair in range(NPAIR):
                if pair == 0:
                    xparts = [(x0c[kc // KCC], kc % KCC) for kc in range(KC1)]
                elif pair == 1:
                    xparts = [(x1_t, kc) for kc in range(KC1)]
                else:
                    x_t = xp.tile([128, KC1, N2], bf16, name=f"x_{pair}",
                                  tag="x")
                    nc.sync.dma_start(
                        out=x_t,
                        in_=x_d[:, pair, :].rearrange("p (k n) -> p k n",
                                                      k=KC1))
                    xparts = [(x_t, kc) for kc in range(KC1)]

                # ---- conv1 + bn1 + relu -> h1 [128, 4, 392] bf16,
                # plus an fp8 copy h1_8 in DoubleRow layout for the g/v
                # projections ----
                h1 = h1p.tile([128, PC, N2], bf16, name=f"h1_{pair}",
                              tag="h1")
                h1_8 = h1p.tile([128, 2, 2, N2P], f8, name=f"h18_{pair}",
                                tag="h18")
                nc.vector.memset(h1_8[:, :, :, N2:], 0.0)
                for oc in range(PC):
                    cps = ps_mm.tile([128, 512], f32, name="cps", tag="mm")
                    for kc in range(KC1):
                        wtile, wi = w1c[kc // KCC], kc % KCC
                        xtile, xi = xparts[kc]
                        nc.tensor.matmul(
                            cps[:, :N2],
                            wtile[:, wi, oc * 128:(oc + 1) * 128],
                            xtile[:, xi, :],
                            start=(kc == 0), stop=(kc == KC1 - 1),
                        )
                    nc.scalar.activation(h1[:, oc, :], cps[:, :N2], Relu,
                                         bias=t1[:, oc:oc + 1])
                    nc.scalar.activation(h1_8[:, oc // 2, oc % 2, :N2],
                                         cps[:, :N2], Relu,
                                         bias=t1[:, oc:oc + 1])

                # ---- score projection g = (Wq^T Wk)^T h1 (fp8 DoubleRow)
                g_sb = qkp.tile([128, PC, N2], bf16, name=f"g_{pair}",
                                tag="g")
                for oc in range(PC):
                    qps = ps_mm.tile([128, 512], f32, name="qps", tag="mm")
                    for dp in range(2):
                        nc.tensor.matmul(
                            qps[:, :N2],
                            mt[:, dp, :, oc * 128:(oc + 1) * 128],
                            h1_8[:, dp, :, :N2],
                            start=(dp == 0), stop=(dp == 1),
                            perf_mode=DR,
                        )
                    nc.vector.tensor_scalar_mul(g_sb[:, oc, :],
                                                qps[:, :N2], 1.0 / 64)

                # ---- per-image attention ----
                vT_list = []
                attnT_list = []
                for j in range(2):
                    # v^T directly: vT[m, c] = sum_p h1[p, m] wvt[p, c]
                    vT = attp.tile([128, 2, P], bf16, name=f"vT_{pair}_{j}",
                                   tag="vT")
                    for mi, (m0, msz) in enumerate(NCHUNKS):
                        vps = ps_mm.tile([128, 512], f32, name="vps",
                                         tag="mm")
                        for dp in range(2):
                            nc.tensor.matmul(
                                vps[:msz, :],
                                h1_8[:, dp, :,
                                     j * N + m0:j * N + m0 + msz],
                                wvt[:, dp, :, :],
                                start=(dp == 0), stop=(dp == 1),
                                perf_mode=DR,
                            )
                        nc.vector.tensor_scalar_mul(vT[:msz, mi, :],
                                                    vps[:msz, :], 1.0 / 32)

                    # attn^T [128, 2, 196] bf16
                    attnT = attp.tile([128, 2, N], bf16,
                                      name=f"aT_{pair}_{j}", tag="attnT")

                    for ni, (n0, nsz) in enumerate(NCHUNKS):
                        lps = ps_sm.tile([128, N], f32, name="lps",
                                         tag="small")
                        # cc: sum_d h1[d, n-slice]^T g[d, :]
                        for pc in range(PC):
                            nc.tensor.matmul(
                                lps[:nsz, :],
                                h1[:, pc, j * N + n0:j * N + n0 + nsz],
                                g_sb[:, pc, j * N:(j + 1) * N],
                                start=(pc == 0), stop=False,
                            )
                        # cp: sum_d P2[d, n-slice]^T h1[d, :]
                        for pc in range(PC):
                            nc.tensor.matmul(
                                lps[:nsz, :],
                                pos[:, pc, n0:n0 + nsz],
                                h1[:, pc, j * N:(j + 1) * N],
                                start=False, stop=(pc == PC - 1),
                            )
                        # softmax over free dim (logits are O(40) max, exp
                        # stays finite in fp32; no max-subtraction needed)
                        p_raw = attp.tile([128, N], f32, name="p_raw",
                                          tag="p_raw")
                        ssum = attp.tile([128, 1], f32, name="ssum",
                                         tag="ss")
                        nc.scalar.activation(p_raw[:nsz, :], lps[:nsz, :],
                                             Exp, accum_out=ssum[:nsz, :])
                        rsum = attp.tile([128, 1], f32, name="rsum",
                                         tag="rs")
                        nc.vector.reciprocal(rsum[:nsz, :], ssum[:nsz, :])
                        p_nrm = attp.tile([128, N], bf16, name="p_nrm",
                                          tag="p_nrm")
                        nc.vector.tensor_scalar_mul(p_nrm[:nsz, :],
                                                    p_raw[:nsz, :],
                                                    rsum[:nsz, :])
                        # transpose normalized attn into attnT[m, n-slice]
                        for mi, (m0, msz) in enumerate(NCHUNKS):
                            tps = ps_tr.tile([128, 128], bf16, name="tps",
                                             tag="tr")
                            nc.tensor.transpose(tps[:msz, :nsz],
                                                p_nrm[:nsz, m0:m0 + msz],
                                                ident[:nsz, :nsz])
                            nc.scalar.activation(
                                attnT[:msz, mi, n0:n0 + nsz],
                                tps[:msz, :nsz], Copy)
                    vT_list.append(vT)
                    attnT_list.append(attnT)

                # ---- attention output + bn2 + relu -> h2 (fp8, DoubleRow
                # layout [dp, i, j*n] padded to 400) ----
                h2 = h2p.tile([128, 2, 2, N2P], f8, name=f"h2_{pair}",
                              tag="h2")
                nc.vector.memset(h2[:, :, :, N2:], 0.0)
                for j in range(2):
                    vT = vT_list[j]
                    attnT = attnT_list[j]
                    for c4 in range(PC):
                        aps = ps_sm.tile([128, N], f32, name="aps",
                                         tag="small")
                        for mi, (m0, msz) in enumerate(NCHUNKS):
                            nc.tensor.matmul(
                                aps[:, :],
                                vT[:msz, mi, c4 * 128:(c4 + 1) * 128],
                                attnT[:msz, mi, :],
                                start=(mi == 0), stop=(mi == 1),
                            )
                        nc.scalar.activation(
                            h2[:, c4 // 2, c4 % 2, j * N:(j + 1) * N],
                            aps[:, :], Relu, bias=t2[:, c4:c4 + 1],
                            scale=s2[:, c4:c4 + 1])

                # ---- conv3 (fp8 DoubleRow) + bn3 + residual + relu -> y ----
                ysb = outp.tile([128, KC1, N2], bf16, name=f"y_{pair}",
                                tag="ysb")
                for oc in range(KC1):
                    ops = ps_mm.tile([128, 512], f32, name="ops", tag="mm")
                    for dp in range(2):
                        nc.tensor.matmul(
                            ops[:, :N2],
                            w3t[:, dp, :, oc * 128:(oc + 1) * 128],
                            h2[:, dp, :, :N2],
                            start=(dp == 0), stop=(dp == 1),
                            perf_mode=DR,
                        )
                    tmp = outp.tile([128, N2], f32, name="tmp", tag="tmp")
                    xtile, xi = xparts[oc]
                    # tmp = conv3/32 + x; w3 is host-scaled by 32 to avoid
                    # fp8 subnormals. bn3 bias lands in the relu below.
                    nc.vector.scalar_tensor_tensor(
                        tmp, ops[:, :N2], 1.0 / 32, xtile[:, xi, :],
                        op0=Mult, op1=Add)
                    nc.scalar.activation(ysb[:, oc, :], tmp, Relu,
                                         bias=t3[:, oc:oc + 1])
                    if oc % 2 == 1:
                        g = oc - 1
                        nc.sync.dma_start(
                            out=y_d[:, pair, g * N2:(g + 2) * N2].rearrange(
                                "p (k n) -> p k n", k=2),
                            in_=ysb[:, g:g + 2, :])

    nc.compile()
    return nc


def _prep_inputs(x, w1, g1, b1, m1, v1, wqkv, rel_h, rel_w,
                 g2, b2, m2, v2, w3, g3, b3, m3, v3):
    f = np.float32
    s1 = (g1 / np.sqrt(v1 + EPS)).astype(f)
    t1 = (b1 - m1 * s1).astype(f)
    s2 = (g2 / np.sqrt(v2 + EPS)).astype(f)
    t2 = (b2 - m2 * s2).astype(f)
    s3 = (g3 / np.sqrt(v3 + EPS)).astype(f)
    t3 = (b3 - m3 * s3).astype(f)

    w1p = (w1 * s1[:, None]).astype(f)                    # [512, 2048]
    w1t = np.ascontiguousarray(
        w1p.T.reshape(KC1, 128, P).transpose(1, 0, 2)).astype(BF16)
    # Fold q/k projections: cc = h1^T (Wq^T Wk) h1 -> g = MT^T h1 with
    # MT[e, d] = (Wk^T Wq)[e, d]; cp = pos^T q -> P2^T h1, P2 = Wq^T pos.
    wq = wqkv[:P].astype(np.float64)                      # [512, 512]
    wk = wqkv[P:2 * P].astype(np.float64)
    # fp8 weights are host-scaled by powers of 2 out of the subnormal
    # range; the inverse scale is folded into on-chip copies.
    # DoubleRow layout: contraction row = dp*256 + i*128 + p.
    mtm = (wk.T @ wq) * 64                                # [512e, 512d]
    mt = np.ascontiguousarray(
        mtm.reshape(2, 2, 128, P).transpose(2, 0, 1, 3)).reshape(
        128, 2 * 2 * P).astype(F8E4)
    wv = wqkv[2 * P:].astype(np.float64) * 32             # [512, 512]
    wvt = np.ascontiguousarray(
        wv.T.reshape(2, 2, 128, P).transpose(2, 0, 1, 3)).reshape(
        128, 2 * 2 * P).astype(F8E4)
    w3p = (w3 * s3[:, None]).astype(f) * 32               # [2048, 512]
    w3t = np.ascontiguousarray(
        w3p.T.reshape(2, 2, 128, CIN).transpose(2, 0, 1, 3)).reshape(
        128, 2 * 2 * CIN).astype(F8E4)
    posm = (np.asarray(rel_h, np.float64)
            + np.asarray(rel_w, np.float64)).reshape(P, N)
    p2 = np.asarray(wqkv[:P], np.float64).T @ posm        # [512d, 196n]
    pos = np.ascontiguousarray(
        p2.reshape(PC, 128, N).transpose(1, 0, 2)).astype(BF16)

    t1_h = np.ascontiguousarray(t1.reshape(PC, 128).T)
    s2_h = np.ascontiguousarray(s2.reshape(PC, 128).T)
    t2_h = np.ascontiguousarray(t2.reshape(PC, 128).T)
    t3_h = np.ascontiguousarray(t3.reshape(KC1, 128).T)

    shared = dict(w1t=w1t, mt=mt, wvt=wvt, w3t=w3t, pos=pos,
                  t1=t1_h, s2=s2_h, t2=t2_h, t3=t3_h)

    xb = np.asarray(x, f).astype(BF16)
    in_maps = []
    for c in range(NCORES):
        # [BPC, CIN, H, W] -> [128, NPAIR, KC1 * N2] with free layout
        # per pair: [kc, j, n]
        xc = xb[c * BPC:(c + 1) * BPC].reshape(NPAIR, 2, KC1, 128, N)
        xc = np.ascontiguousarray(xc.transpose(3, 0, 2, 1, 4)).reshape(
            128, NPAIR, KC1 * N2)
        in_maps.append(dict(shared, x=xc))
    return in_maps


def _run(in_maps, trace=False):
    from concourse.bass_utils import run_bass_kernel_spmd
    if "nc" not in _CACHE:
        _CACHE["nc"] = _build()
    nc = _CACHE["nc"]
    return run_bass_kernel_spmd(nc, in_maps, core_ids=list(range(NCORES)),
                                trace=trace)


def _assemble(results):
    out = np.empty((B, CIN, H, W), np.float32)
    for c in range(NCORES):
        # [128, NPAIR, KC1*N2] -> [BPC, CIN, H, W]
        yc = results[c]["y"].reshape(128, NPAIR, KC1, 2, N).astype(
            np.float32)
        out[c * BPC:(c + 1) * BPC] = yc.transpose(1, 3, 2, 0, 4).reshape(
            BPC, CIN, H, W)
    return out


def kernel(**inputs):
    in_maps = _prep_inputs(**inputs)
    res = _run(in_maps)
    return _assemble(res.results)


# revision 35
# speedup vs baseline: 1.5600x; 1.0654x over previous
"""Trainium2 Bass kernel for the MHSA bottleneck block.

Contract: kernel(**inputs) takes the FULL unsharded inputs (as produced by
setup_inputs()) and returns the FULL [64, 2048, 14, 14] float32 output.
Internally shards data-parallel over batch: 8 images per NeuronCore, 8 cores.

Precision plan (error budget 2e-2, lands ~3e-3):
  - conv1 / attention matmuls: bf16 operands, fp32 PSUM accumulate.
  - conv3: fp8e4 DoubleRow (2 rows/cycle), h2 + w3 in fp8. conv3's output
    is small relative to the residual x, so fp8 noise there is damped.
Attention-score algebra: cc = q^T k = h1^T (Wq^T Wk) h1, so only one
projection g = (Wq^T Wk)^T h1 is computed on-chip; cp = pos^T q folds to
P2^T h1 with P2 = Wq^T pos precomputed on host. This removes the q/k
projections entirely.
DMA issue order is arranged so the first conv1 matmul only waits on the
first quarter of x(pair0) + w1t.
"""
import sys

sys.path.insert(0, '/opt/trn_rl_repo')

import numpy as np
import ml_dtypes

BF16 = ml_dtypes.bfloat16
F8E4 = ml_dtypes.float8_e4m3

# Problem constants (hardcoded per the harness contract).
B, CIN, P, H, W = 64, 2048, 512, 14, 14
EPS = 1e-5
N = H * W            # 196 pixels
NCORES = 8
BPC = B // NCORES    # 8 images per core
NPAIR = BPC // 2     # 4 image pairs per core
KC1 = CIN // 128     # 16 input-channel chunks for conv1 / output chunks conv3
PC = P // 128        # 4 chunks of the 512-dim
N2 = 2 * N           # 392 = free dim for image-pair matmuls
N2P = 400            # N2 padded so fp8 DoubleRow strides are 16B-aligned

# n/m chunking of the 196-pixel dim: 128 + 68
NCHUNKS = [(0, 128), (128, 68)]

_CACHE = {}


def _build():
    import concourse.bass as bass  # noqa: F401
    import concourse.mybir as mybir
    import concourse.tile as tile
    from concourse import bacc
    from concourse.masks import make_identity

    f32 = mybir.dt.float32
    bf16 = mybir.dt.bfloat16
    f8 = mybir.dt.float8e4
    DR = mybir.MatmulPerfMode.DoubleRow

    nc = bacc.Bacc(None, target_bir_lowering=False, debug=False)

    # DRAM parameters, partition-major so each DMA is one long line per
    # partition.
    x_d = nc.declare_dram_parameter("x", [128, NPAIR, KC1 * N2], bf16,
                                    isOutput=False)
    w1t_d = nc.declare_dram_parameter("w1t", [128, KC1, P], bf16,
                                      isOutput=False)
    mt_d = nc.declare_dram_parameter("mt", [128, 2 * 2 * P], f8,
                                     isOutput=False)
    wvt_d = nc.declare_dram_parameter("wvt", [128, 2 * 2 * P], f8,
                                      isOutput=False)
    w3t_d = nc.declare_dram_parameter("w3t", [128, 2 * 2 * CIN], f8,
                                      isOutput=False)
    pos_d = nc.declare_dram_parameter("pos", [128, PC, N], bf16,
                                      isOutput=False)
    t1_d = nc.declare_dram_parameter("t1", [128, PC], f32, isOutput=False)
    s2_d = nc.declare_dram_parameter("s2", [128, PC], f32, isOutput=False)
    t2_d = nc.declare_dram_parameter("t2", [128, PC], f32, isOutput=False)
    t3_d = nc.declare_dram_parameter("t3", [128, KC1], f32, isOutput=False)
    y_d = nc.declare_dram_parameter("y", [128, NPAIR, KC1 * N2], bf16,
                                    isOutput=True)

    XCH = 8              # x/w1 startup chunks for pair 0
    KCC = KC1 // XCH     # 2 kc per chunk

    with tile.TileContext(nc) as tc:
        with (
            tc.tile_pool(name="const", bufs=1) as const,
            tc.tile_pool(name="xp", bufs=3) as xp,
            tc.tile_pool(name="x0p", bufs=1) as x0p,
            tc.tile_pool(name="h1p", bufs=2) as h1p,
            tc.tile_pool(name="qkp", bufs=2) as qkp,
            tc.tile_pool(name="h2p", bufs=2) as h2p,
            tc.tile_pool(name="attp", bufs=2) as attp,
            tc.tile_pool(name="outp", bufs=2) as outp,
            tc.tile_pool(name="ps_mm", bufs=4, space="PSUM") as ps_mm,
            tc.tile_pool(name="ps_sm", bufs=3, space="PSUM") as ps_sm,
            tc.tile_pool(name="ps_tr", bufs=1, space="PSUM") as ps_tr,
        ):
            # ---- startup: interleave x(pair0) and w1t quarter-chunks so the
            # first conv1 matmul waits on only 1/4 of each ----
            x0c = []
            w1c = []
            for c in range(XCH):
                xt = x0p.tile([128, KCC, N2], bf16, name=f"x0_{c}")
                nc.sync.dma_start(
                    out=xt,
                    in_=x_d[:, 0, c * KCC * N2:(c + 1) * KCC * N2].rearrange(
                        "p (k n) -> p k n", k=KCC))
                x0c.append(xt)
                wt = const.tile([128, KCC, P], bf16, name=f"w1_{c}")
                nc.sync.dma_start(out=wt, in_=w1t_d[:, c * KCC:(c + 1) * KCC, :])
                w1c.append(wt)
            t1 = const.tile([128, PC], f32)
            nc.sync.dma_start(out=t1, in_=t1_d[:, :])
            mt = const.tile([128, 2, 2, P], f8)
            nc.sync.dma_start(
                out=mt,
                in_=mt_d[:, :].rearrange("p (a b c) -> p a b c", a=2, b=2))
            wvt = const.tile([128, 2, 2, P], f8)
            nc.sync.dma_start(
                out=wvt,
                in_=wvt_d[:, :].rearrange("p (a b c) -> p a b c", a=2, b=2))
            pos = const.tile([128, PC, N], bf16)
            nc.sync.dma_start(out=pos, in_=pos_d[:, :, :])
            s2 = const.tile([128, PC], f32)
            nc.sync.dma_start(out=s2, in_=s2_d[:, :])
            t2 = const.tile([128, PC], f32)
            nc.sync.dma_start(out=t2, in_=t2_d[:, :])
            # x(pair1) ahead of w3 so pair1's conv1 isn't DMA-gated
            x1_t = xp.tile([128, KC1, N2], bf16, name="x_1", tag="x")
            nc.sync.dma_start(
                out=x1_t,
                in_=x_d[:, 1, :].rearrange("p (k n) -> p k n", k=KC1))
            w3t = const.tile([128, 2, 2, CIN], f8)
            nc.sync.dma_start(
                out=w3t,
                in_=w3t_d[:, :].rearrange("p (a b c) -> p a b c", a=2, b=2))
            t3 = const.tile([128, KC1], f32)
            nc.sync.dma_start(out=t3, in_=t3_d[:, :])

            ident = const.tile([128, 128], bf16)
            make_identity(nc, ident)

            Exp = mybir.ActivationFunctionType.Exp
            Relu = mybir.ActivationFunctionType.Relu
            Copy = mybir.ActivationFunctionType.Copy
            Add = mybir.AluOpType.add
            Mult = mybir.AluOpType.mult
            Max = mybir.AluOpType.max

            # per-pair x access plans; pair 0/1 tiles were DMA'd above
            xparts_all = {
                0: [(x0c[kc // KCC], kc % KCC) for kc in range(KC1)],
                1: [(x1_t, kc) for kc in range(KC1)],
            }

            def fetch_x(pair):
                if pair in xparts_all:
                    return
                x_t = xp.tile([128, KC1, N2], bf16, name=f"x_{pair}",
                              tag="x")
                nc.sync.dma_start(
                    out=x_t,
                    in_=x_d[:, pair, :].rearrange("p (k n) -> p k n",
                                                  k=KC1))
                xparts_all[pair] = [(x_t, kc) for kc in range(KC1)]

            h1_tiles = {}

            def conv1_oc(pair, oc):
                # one conv1 output-chunk: matmuls + bn1/relu into h1 (bf16)
                # and an fp8 DoubleRow-layout copy for the g/v projections
                h1, h1_8 = h1_tiles[pair]
                xparts = xparts_all[pair]
                cps = ps_mm.tile([128, 512], f32, name="cps", tag="mm")
                for kc in range(KC1):
                    wtile, wi = w1c[kc // KCC], kc % KCC
                    xtile, xi = xparts[kc]
                    nc.tensor.matmul(
                        cps[:, :N2],
                        wtile[:, wi, oc * 128:(oc + 1) * 128],
                        xtile[:, xi, :],
                        start=(kc == 0), stop=(kc == KC1 - 1),
                    )
                nc.scalar.activation(h1[:, oc, :], cps[:, :N2], Relu,
                                     bias=t1[:, oc:oc + 1])
                nc.vector.tensor_copy(h1_8[:, oc // 2, oc % 2, :N2],
                                      h1[:, oc, :])

            def conv1_alloc(pair):
                h1 = h1p.tile([128, PC, N2], bf16, name=f"h1_{pair}",
                              tag="h1")
                h1_8 = h1p.tile([128, 2, 2, N2P], f8, name=f"h18_{pair}",
                                tag="h18")
                nc.vector.memset(h1_8[:, :, :, N2:], 0.0)
                h1_tiles[pair] = (h1, h1_8)

            # pair 0's conv1 runs standalone (gated by the startup DMAs);
            # conv1 of pair p+1 is emitted inside pair p's attention to fill
            # the PE while softmax chains resolve.
            conv1_alloc(0)
            for oc in range(PC):
                conv1_oc(0, oc)

            for pair in range(NPAIR):
                h1, h1_8 = h1_tiles[pair]
                nxt = pair + 1 if pair + 1 < NPAIR else None
                if nxt is not None:
                    fetch_x(nxt)
                    conv1_alloc(nxt)
                filler = list(range(PC)) if nxt is not None else []

                # ---- score projection g = (Wq^T Wk)^T h1 (fp8 DoubleRow)
                g_sb = qkp.tile([128, PC, N2], bf16, name=f"g_{pair}",
                                tag="g")
                for oc in range(PC):
                    qps = ps_mm.tile([128, 512], f32, name="qps", tag="mm")
                    for dp in range(2):
                        nc.tensor.matmul(
                            qps[:, :N2],
                            mt[:, dp, :, oc * 128:(oc + 1) * 128],
                            h1_8[:, dp, :, :N2],
                            start=(dp == 0), stop=(dp == 1),
                            perf_mode=DR,
                        )
                    nc.vector.tensor_scalar_mul(g_sb[:, oc, :],
                                                qps[:, :N2], 1.0 / 64)

                # ---- per-image attention ----
                vT_list = []
                attnT_list = []
                for j in range(2):
                    # v^T directly: vT[m, c] = sum_p h1[p, m] wvt[p, c]
                    vT = attp.tile([128, 2, P], bf16, name=f"vT_{pair}_{j}",
                                   tag="vT")
                    for mi, (m0, msz) in enumerate(NCHUNKS):
                        vps = ps_mm.tile([128, 512], f32, name="vps",
                                         tag="mm")
                        for dp in range(2):
                            nc.tensor.matmul(
                                vps[:msz, :],
                                h1_8[:, dp, :,
                                     j * N + m0:j * N + m0 + msz],
                                wvt[:, dp, :, :],
                                start=(dp == 0), stop=(dp == 1),
                                perf_mode=DR,
                            )
                        nc.vector.tensor_scalar_mul(vT[:msz, mi, :],
                                                    vps[:msz, :], 1.0 / 32)
                    vT_list.append(vT)

                for j in range(2):
                    # attn^T [128, 2, 196] bf16
                    attnT = attp.tile([128, 2, N], bf16,
                                      name=f"aT_{pair}_{j}", tag="attnT")

                    for ni, (n0, nsz) in enumerate(NCHUNKS):
                        lps = ps_sm.tile([128, N], f32, name="lps",
                                         tag="small")
                        # cc: sum_d h1[d, n-slice]^T g[d, :]
                        for pc in range(PC):
                            nc.tensor.matmul(
                                lps[:nsz, :],
                                h1[:, pc, j * N + n0:j * N + n0 + nsz],
                                g_sb[:, pc, j * N:(j + 1) * N],
                                start=(pc == 0), stop=False,
                            )
                        # cp: sum_d P2[d, n-slice]^T h1[d, :]
                        for pc in range(PC):
                            nc.tensor.matmul(
                                lps[:nsz, :],
                                pos[:, pc, n0:n0 + nsz],
                                h1[:, pc, j * N:(j + 1) * N],
                                start=False, stop=(pc == PC - 1),
                            )
                        # softmax over free dim (logits are O(40) max, exp
                        # stays finite in fp32; no max-subtraction needed)
                        p_raw = attp.tile([128, N], f32, name="p_raw",
                                          tag="p_raw")
                        ssum = attp.tile([128, 1], f32, name="ssum",
                                         tag="ss")
                        nc.scalar.activation(p_raw[:nsz, :], lps[:nsz, :],
                                             Exp, accum_out=ssum[:nsz, :])
                        rsum = attp.tile([128, 1], f32, name="rsum",
                                         tag="rs")
                        nc.vector.reciprocal(rsum[:nsz, :], ssum[:nsz, :])
                        p_nrm = attp.tile([128, N], bf16, name="p_nrm",
                                          tag="p_nrm")
                        nc.vector.tensor_scalar_mul(p_nrm[:nsz, :],
                                                    p_raw[:nsz, :],
                                                    rsum[:nsz, :])
                        # fill the PE with a conv1 chunk of the next pair
                        # while the softmax chain resolves
                        if filler:
                            conv1_oc(nxt, filler.pop(0))
                        # transpose normalized attn into attnT[m, n-slice]
                        for mi, (m0, msz) in enumerate(NCHUNKS):
                            tps = ps_tr.tile([128, 128], bf16, name="tps",
                                             tag="tr")
                            nc.tensor.transpose(tps[:msz, :nsz],
                                                p_nrm[:nsz, m0:m0 + msz],
                                                ident[:nsz, :nsz])
                            nc.scalar.activation(
                                attnT[:msz, mi, n0:n0 + nsz],
                                tps[:msz, :nsz], Copy)
                    attnT_list.append(attnT)

                # ---- attention output + bn2 + relu -> h2 (fp8, DoubleRow
                # layout [dp, i, j*n] padded to 400) ----
                h2 = h2p.tile([128, 2, 2, N2P], f8, name=f"h2_{pair}",
                              tag="h2")
                nc.vector.memset(h2[:, :, :, N2:], 0.0)
                for j in range(2):
                    vT = vT_list[j]
                    attnT = attnT_list[j]
                    for c4 in range(PC):
                        aps = ps_sm.tile([128, N], f32, name="aps",
                                         tag="small")
                        for mi, (m0, msz) in enumerate(NCHUNKS):
                            nc.tensor.matmul(
                                aps[:, :],
                                vT[:msz, mi, c4 * 128:(c4 + 1) * 128],
                                attnT[:msz, mi, :],
                                start=(mi == 0), stop=(mi == 1),
                            )
                        nc.scalar.activation(
                            h2[:, c4 // 2, c4 % 2, j * N:(j + 1) * N],
                            aps[:, :], Relu, bias=t2[:, c4:c4 + 1],
                            scale=s2[:, c4:c4 + 1])

                # ---- conv3 (fp8 DoubleRow) + bn3 + residual + relu -> y ----
                ysb = outp.tile([128, KC1, N2], bf16, name=f"y_{pair}",
                                tag="ysb")
                for oc in range(KC1):
                    ops = ps_mm.tile([128, 512], f32, name="ops", tag="mm")
                    for dp in range(2):
                        nc.tensor.matmul(
                            ops[:, :N2],
                            w3t[:, dp, :, oc * 128:(oc + 1) * 128],
                            h2[:, dp, :, :N2],
                            start=(dp == 0), stop=(dp == 1),
                            perf_mode=DR,
                        )
                    tmp = outp.tile([128, N2], f32, name="tmp", tag="tmp")
                    xtile, xi = xparts_all[pair][oc]
                    # tmp = conv3/32 + x; w3 is host-scaled by 32 to avoid
                    # fp8 subnormals. bn3 bias lands in the relu below.
                    nc.vector.scalar_tensor_tensor(
                        tmp, ops[:, :N2], 1.0 / 32, xtile[:, xi, :],
                        op0=Mult, op1=Add)
                    if oc % 4 == 1:
                        # spread relu load: some chunks on the vector engine
                        nc.vector.tensor_scalar(ysb[:, oc, :], tmp,
                                                t3[:, oc:oc + 1], 0.0,
                                                op0=Add, op1=Max)
                    else:
                        nc.scalar.activation(ysb[:, oc, :], tmp, Relu,
                                             bias=t3[:, oc:oc + 1])
                    if oc % 2 == 1:
                        g = oc - 1
                        nc.sync.dma_start(
                            out=y_d[:, pair, g * N2:(g + 2) * N2].rearrange(
                                "p (k n) -> p k n", k=2),
                            in_=ysb[:, g:g + 2, :])

    nc.compile()
    return nc


def _prep_inputs(x, w1, g1, b1, m1, v1, wqkv, rel_h, rel_w,
                 g2, b2, m2, v2, w3, g3, b3, m3, v3):
    f = np.float32
    s1 = (g1 / np.sqrt(v1 + EPS)).astype(f)
    t1 = (b1 - m1 * s1).astype(f)
    s2 = (g2 / np.sqrt(v2 + EPS)).astype(f)
    t2 = (b2 - m2 * s2).astype(f)
    s3 = (g3 / np.sqrt(v3 + EPS)).astype(f)
    t3 = (b3 - m3 * s3).astype(f)

    w1p = (w1 * s1[:, None]).astype(f)                    # [512, 2048]
    w1t = np.ascontiguousarray(
        w1p.T.reshape(KC1, 128, P).transpose(1, 0, 2)).astype(BF16)
    # Fold q/k projections: cc = h1^T (Wq^T Wk) h1 -> g = MT^T h1 with
    # MT[e, d] = (Wk^T Wq)[e, d]; cp = pos^T q -> P2^T h1, P2 = Wq^T pos.
    wq = wqkv[:P].astype(np.float64)                      # [512, 512]
    wk = wqkv[P:2 * P].astype(np.float64)
    # fp8 weights are host-scaled by powers of 2 out of the subnormal
    # range; the inverse scale is folded into on-chip copies.
    # DoubleRow layout: contraction row = dp*256 + i*128 + p.
    mtm = (wk.T @ wq) * 64                                # [512e, 512d]
    mt = np.ascontiguousarray(
        mtm.reshape(2, 2, 128, P).transpose(2, 0, 1, 3)).reshape(
        128, 2 * 2 * P).astype(F8E4)
    wv = wqkv[2 * P:].astype(np.float64) * 32             # [512, 512]
    wvt = np.ascontiguousarray(
        wv.T.reshape(2, 2, 128, P).transpose(2, 0, 1, 3)).reshape(
        128, 2 * 2 * P).astype(F8E4)
    w3p = (w3 * s3[:, None]).astype(f) * 32               # [2048, 512]
    w3t = np.ascontiguousarray(
        w3p.T.reshape(2, 2, 128, CIN).transpose(2, 0, 1, 3)).reshape(
        128, 2 * 2 * CIN).astype(F8E4)
    posm = (np.asarray(rel_h, np.float64)
            + np.asarray(rel_w, np.float64)).reshape(P, N)
    p2 = np.asarray(wqkv[:P], np.float64).T @ posm        # [512d, 196n]
    pos = np.ascontiguousarray(
        p2.reshape(PC, 128, N).transpose(1, 0, 2)).astype(BF16)

    t1_h = np.ascontiguousarray(t1.reshape(PC, 128).T)
    s2_h = np.ascontiguousarray(s2.reshape(PC, 128).T)
    t2_h = np.ascontiguousarray(t2.reshape(PC, 128).T)
    t3_h = np.ascontiguousarray(t3.reshape(KC1, 128).T)

    shared = dict(w1t=w1t, mt=mt, wvt=wvt, w3t=w3t, pos=pos,
                  t1=t1_h, s2=s2_h, t2=t2_h, t3=t3_h)

    xb = np.asarray(x, f).astype(BF16)
    in_maps = []
    for c in range(NCORES):
        # [BPC, CIN, H, W] -> [128, NPAIR, KC1 * N2] with free layout
        # per pair: [kc, j, n]
        xc = xb[c * BPC:(c + 1) * BPC].reshape(NPAIR, 2, KC1, 128, N)
        xc = np.ascontiguousarray(xc.transpose(3, 0, 2, 1, 4)).reshape(
            128, NPAIR, KC1 * N2)
        in_maps.append(dict(shared, x=xc))
    return in_maps


def _run(in_maps, trace=False):
    from concourse.bass_utils import run_bass_kernel_spmd
    if "nc" not in _CACHE:
        _CACHE["nc"] = _build()
    nc = _CACHE["nc"]
    return run_bass_kernel_spmd(nc, in_maps, core_ids=list(range(NCORES)),
                                trace=trace)


def _assemble(results):
    out = np.empty((B, CIN, H, W), np.float32)
    for c in range(NCORES):
        # [128, NPAIR, KC1*N2] -> [BPC, CIN, H, W]
        yc = results[c]["y"].reshape(128, NPAIR, KC1, 2, N).astype(
            np.float32)
        out[c * BPC:(c + 1) * BPC] = yc.transpose(1, 3, 2, 0, 4).reshape(
            BPC, CIN, H, W)
    return out


def kernel(**inputs):
    in_maps = _prep_inputs(**inputs)
    res = _run(in_maps)
    return _assemble(res.results)


# revision 38
# speedup vs baseline: 1.7077x; 1.0946x over previous
"""Trainium2 Bass kernel for the MHSA bottleneck block.

Contract: kernel(**inputs) takes the FULL unsharded inputs (as produced by
setup_inputs()) and returns the FULL [64, 2048, 14, 14] float32 output.
Internally shards data-parallel over batch: 8 images per NeuronCore, 8 cores.

Precision plan (error budget 2e-2, lands ~3e-3):
  - conv1 / attention matmuls: bf16 operands, fp32 PSUM accumulate.
  - conv3: fp8e4 DoubleRow (2 rows/cycle), h2 + w3 in fp8. conv3's output
    is small relative to the residual x, so fp8 noise there is damped.
Attention-score algebra: cc = q^T k = h1^T (Wq^T Wk) h1, so only one
projection g = (Wq^T Wk)^T h1 is computed on-chip; cp = pos^T q folds to
P2^T h1 with P2 = Wq^T pos precomputed on host. This removes the q/k
projections entirely.
DMA issue order is arranged so the first conv1 matmul only waits on the
first quarter of x(pair0) + w1t.
"""
import sys

sys.path.insert(0, '/opt/trn_rl_repo')

import numpy as np
import ml_dtypes

BF16 = ml_dtypes.bfloat16
F8E4 = ml_dtypes.float8_e4m3

# Problem constants (hardcoded per the harness contract).
B, CIN, P, H, W = 64, 2048, 512, 14, 14
EPS = 1e-5
N = H * W            # 196 pixels
NCORES = 8
BPC = B // NCORES    # 8 images per core
NPAIR = BPC // 2     # 4 image pairs per core
KC1 = CIN // 128     # 16 input-channel chunks for conv1 / output chunks conv3
PC = P // 128        # 4 chunks of the 512-dim
N2 = 2 * N           # 392 = free dim for image-pair matmuls
N2P = 400            # N2 padded so fp8 DoubleRow strides are 16B-aligned

# n/m chunking of the 196-pixel dim: 128 + 68
NCHUNKS = [(0, 128), (128, 68)]

_CACHE = {}


def _build():
    import concourse.bass as bass  # noqa: F401
    import concourse.mybir as mybir
    import concourse.tile as tile
    from concourse import bacc
    from concourse.masks import make_identity

    f32 = mybir.dt.float32
    bf16 = mybir.dt.bfloat16
    f8 = mybir.dt.float8e4
    DR = mybir.MatmulPerfMode.DoubleRow

    nc = bacc.Bacc(None, target_bir_lowering=False, debug=False)

    # DRAM parameters, partition-major so each DMA is one long line per
    # partition.
    x_d = nc.declare_dram_parameter("x", [128, NPAIR, KC1 * N2], bf16,
                                    isOutput=False)
    w1t_d = nc.declare_dram_parameter("w1t", [128, KC1, P], bf16,
                                      isOutput=False)
    mt_d = nc.declare_dram_parameter("mt", [128, 2 * 2 * P], f8,
                                     isOutput=False)
    wvt_d = nc.declare_dram_parameter("wvt", [128, 2 * 2 * P], f8,
                                      isOutput=False)
    w3t_d = nc.declare_dram_parameter("w3t", [128, 2 * 2 * CIN], f8,
                                      isOutput=False)
    pos_d = nc.declare_dram_parameter("pos", [128, PC, N], bf16,
                                      isOutput=False)
    t1_d = nc.declare_dram_parameter("t1", [128, PC], f32, isOutput=False)
    s2_d = nc.declare_dram_parameter("s2", [128, PC], f32, isOutput=False)
    t2_d = nc.declare_dram_parameter("t2", [128, PC], f32, isOutput=False)
    t3_d = nc.declare_dram_parameter("t3", [128, KC1], f32, isOutput=False)
    y_d = nc.declare_dram_parameter("y", [128, NPAIR, KC1 * N2], bf16,
                                    isOutput=True)

    XCH = 8              # x/w1 startup chunks for pair 0
    KCC = KC1 // XCH     # 2 kc per chunk

    with tile.TileContext(nc) as tc:
        with (
            tc.tile_pool(name="const", bufs=1) as const,
            tc.tile_pool(name="xp", bufs=3) as xp,
            tc.tile_pool(name="x0p", bufs=1) as x0p,
            tc.tile_pool(name="h1p", bufs=2) as h1p,
            tc.tile_pool(name="qkp", bufs=2) as qkp,
            tc.tile_pool(name="h2p", bufs=2) as h2p,
            tc.tile_pool(name="attp", bufs=2) as attp,
            tc.tile_pool(name="outp", bufs=2) as outp,
            tc.tile_pool(name="ps_mm", bufs=4, space="PSUM") as ps_mm,
            tc.tile_pool(name="ps_sm", bufs=3, space="PSUM") as ps_sm,
            tc.tile_pool(name="ps_tr", bufs=1, space="PSUM") as ps_tr,
        ):
            # ---- startup: interleave x(pair0) and w1t quarter-chunks so the
            # first conv1 matmul waits on only 1/4 of each ----
            x0c = []
            w1c = []
            for c in range(XCH):
                xt = x0p.tile([128, KCC, N2], bf16, name=f"x0_{c}")
                nc.sync.dma_start(
                    out=xt,
                    in_=x_d[:, 0, c * KCC * N2:(c + 1) * KCC * N2].rearrange(
                        "p (k n) -> p k n", k=KCC))
                x0c.append(xt)
                wt = const.tile([128, KCC, P], bf16, name=f"w1_{c}")
                nc.sync.dma_start(out=wt, in_=w1t_d[:, c * KCC:(c + 1) * KCC, :])
                w1c.append(wt)
            t1 = const.tile([128, PC], f32)
            nc.sync.dma_start(out=t1, in_=t1_d[:, :])
            mt = const.tile([128, 2, 2, P], f8)
            nc.sync.dma_start(
                out=mt,
                in_=mt_d[:, :].rearrange("p (a b c) -> p a b c", a=2, b=2))
            wvt = const.tile([128, 2, 2, P], f8)
            nc.sync.dma_start(
                out=wvt,
                in_=wvt_d[:, :].rearrange("p (a b c) -> p a b c", a=2, b=2))
            pos = const.tile([128, PC, N], bf16)
            nc.sync.dma_start(out=pos, in_=pos_d[:, :, :])
            s2 = const.tile([128, PC], f32)
            nc.sync.dma_start(out=s2, in_=s2_d[:, :])
            t2 = const.tile([128, PC], f32)
            nc.sync.dma_start(out=t2, in_=t2_d[:, :])
            # x(pair1) ahead of w3 so pair1's conv1 isn't DMA-gated
            x1_t = xp.tile([128, KC1, N2], bf16, name="x_1", tag="x")
            nc.sync.dma_start(
                out=x1_t,
                in_=x_d[:, 1, :].rearrange("p (k n) -> p k n", k=KC1))
            w3t = const.tile([128, 2, 2, CIN], f8)
            nc.sync.dma_start(
                out=w3t,
                in_=w3t_d[:, :].rearrange("p (a b c) -> p a b c", a=2, b=2))
            t3 = const.tile([128, KC1], f32)
            nc.sync.dma_start(out=t3, in_=t3_d[:, :])

            ident = const.tile([128, 128], bf16)
            make_identity(nc, ident)
            # 32*I: adds the residual x into conv3's PSUM, pre-scaled to
            # match the w3 fp8 host-scale of 32 (undone in the relu).
            ident32 = const.tile([128, 128], bf16)
            nc.gpsimd.memset(ident32, 0.0)
            nc.gpsimd.affine_select(
                out=ident32, in_=ident32,
                compare_op=mybir.AluOpType.not_equal, fill=32.0, base=0,
                pattern=[[-1, 128]], channel_multiplier=1)

            Exp = mybir.ActivationFunctionType.Exp
            Relu = mybir.ActivationFunctionType.Relu
            Copy = mybir.ActivationFunctionType.Copy
            Add = mybir.AluOpType.add
            Mult = mybir.AluOpType.mult
            Max = mybir.AluOpType.max

            # per-pair x access plans; pair 0/1 tiles were DMA'd above
            xparts_all = {
                0: [(x0c[kc // KCC], kc % KCC) for kc in range(KC1)],
                1: [(x1_t, kc) for kc in range(KC1)],
            }

            def fetch_x(pair):
                if pair in xparts_all:
                    return
                x_t = xp.tile([128, KC1, N2], bf16, name=f"x_{pair}",
                              tag="x")
                nc.sync.dma_start(
                    out=x_t,
                    in_=x_d[:, pair, :].rearrange("p (k n) -> p k n",
                                                  k=KC1))
                xparts_all[pair] = [(x_t, kc) for kc in range(KC1)]

            h1_tiles = {}

            def conv1_oc(pair, oc):
                # one conv1 output-chunk: matmuls + bn1/relu into h1 (bf16)
                # and an fp8 DoubleRow-layout copy for the g/v projections
                h1, h1_8 = h1_tiles[pair]
                xparts = xparts_all[pair]
                cps = ps_mm.tile([128, 512], f32, name="cps", tag="mm")
                for kc in range(KC1):
                    wtile, wi = w1c[kc // KCC], kc % KCC
                    xtile, xi = xparts[kc]
                    nc.tensor.matmul(
                        cps[:, :N2],
                        wtile[:, wi, oc * 128:(oc + 1) * 128],
                        xtile[:, xi, :],
                        start=(kc == 0), stop=(kc == KC1 - 1),
                    )
                nc.scalar.activation(h1[:, oc, :], cps[:, :N2], Relu,
                                     bias=t1[:, oc:oc + 1])
                nc.vector.tensor_copy(h1_8[:, oc // 2, oc % 2, :N2],
                                      h1[:, oc, :])

            def conv1_alloc(pair):
                h1 = h1p.tile([128, PC, N2], bf16, name=f"h1_{pair}",
                              tag="h1")
                h1_8 = h1p.tile([128, 2, 2, N2P], f8, name=f"h18_{pair}",
                                tag="h18")
                nc.vector.memset(h1_8[:, :, :, N2:], 0.0)
                h1_tiles[pair] = (h1, h1_8)

            # pair 0's conv1 runs standalone (gated by the startup DMAs);
            # conv1 of pair p+1 is emitted inside pair p's attention to fill
            # the PE while softmax chains resolve.
            conv1_alloc(0)
            for oc in range(PC):
                conv1_oc(0, oc)

            for pair in range(NPAIR):
                h1, h1_8 = h1_tiles[pair]
                nxt = pair + 1 if pair + 1 < NPAIR else None
                if nxt is not None:
                    fetch_x(nxt)
                    conv1_alloc(nxt)
                filler = list(range(PC)) if nxt is not None else []

                # ---- score projection g = (Wq^T Wk)^T h1 (fp8 DoubleRow)
                g_sb = qkp.tile([128, PC, N2], bf16, name=f"g_{pair}",
                                tag="g")
                for oc in range(PC):
                    qps = ps_mm.tile([128, 512], f32, name="qps", tag="mm")
                    for dp in range(2):
                        nc.tensor.matmul(
                            qps[:, :N2],
                            mt[:, dp, :, oc * 128:(oc + 1) * 128],
                            h1_8[:, dp, :, :N2],
                            start=(dp == 0), stop=(dp == 1),
                            perf_mode=DR,
                        )
                    nc.vector.tensor_scalar_mul(g_sb[:, oc, :],
                                                qps[:, :N2], 1.0 / 64)

                # ---- per-image attention ----
                vT_list = []
                attnT_list = []
                for j in range(2):
                    # v^T directly: vT[m, c] = sum_p h1[p, m] wvt[p, c]
                    vT = attp.tile([128, 2, P], bf16, name=f"vT_{pair}_{j}",
                                   tag="vT")
                    for mi, (m0, msz) in enumerate(NCHUNKS):
                        vps = ps_mm.tile([128, 512], f32, name="vps",
                                         tag="mm")
                        for dp in range(2):
                            nc.tensor.matmul(
                                vps[:msz, :],
                                h1_8[:, dp, :,
                                     j * N + m0:j * N + m0 + msz],
                                wvt[:, dp, :, :],
                                start=(dp == 0), stop=(dp == 1),
                                perf_mode=DR,
                            )
                        nc.vector.tensor_scalar_mul(vT[:msz, mi, :],
                                                    vps[:msz, :], 1.0 / 32)
                    vT_list.append(vT)

                for j in range(2):
                    # attn^T [128, 2, 196] bf16
                    attnT = attp.tile([128, 2, N], bf16,
                                      name=f"aT_{pair}_{j}", tag="attnT")

                    for ni, (n0, nsz) in enumerate(NCHUNKS):
                        lps = ps_sm.tile([128, N], f32, name="lps",
                                         tag="small")
                        # cc: sum_d h1[d, n-slice]^T g[d, :]
                        for pc in range(PC):
                            nc.tensor.matmul(
                                lps[:nsz, :],
                                h1[:, pc, j * N + n0:j * N + n0 + nsz],
                                g_sb[:, pc, j * N:(j + 1) * N],
                                start=(pc == 0), stop=False,
                            )
                        # cp: sum_d P2[d, n-slice]^T h1[d, :]
                        for pc in range(PC):
                            nc.tensor.matmul(
                                lps[:nsz, :],
                                pos[:, pc, n0:n0 + nsz],
                                h1[:, pc, j * N:(j + 1) * N],
                                start=False, stop=(pc == PC - 1),
                            )
                        # softmax over free dim (logits are O(40) max, exp
                        # stays finite in fp32; no max-subtraction needed)
                        p_raw = attp.tile([128, N], f32, name="p_raw",
                                          tag="p_raw")
                        ssum = attp.tile([128, 1], f32, name="ssum",
                                         tag="ss")
                        nc.scalar.activation(p_raw[:nsz, :], lps[:nsz, :],
                                             Exp, accum_out=ssum[:nsz, :])
                        rsum = attp.tile([128, 1], f32, name="rsum",
                                         tag="rs")
                        nc.vector.reciprocal(rsum[:nsz, :], ssum[:nsz, :])
                        p_nrm = attp.tile([128, N], bf16, name="p_nrm",
                                          tag="p_nrm")
                        nc.vector.tensor_scalar_mul(p_nrm[:nsz, :],
                                                    p_raw[:nsz, :],
                                                    rsum[:nsz, :])
                        # fill the PE with a conv1 chunk of the next pair
                        # while the softmax chain resolves
                        if filler:
                            conv1_oc(nxt, filler.pop(0))
                        # transpose normalized attn into attnT[m, n-slice]
                        for mi, (m0, msz) in enumerate(NCHUNKS):
                            tps = ps_tr.tile([128, 128], bf16, name="tps",
                                             tag="tr")
                            nc.tensor.transpose(tps[:msz, :nsz],
                                                p_nrm[:nsz, m0:m0 + msz],
                                                ident[:nsz, :nsz])
                            nc.vector.tensor_copy(
                                attnT[:msz, mi, n0:n0 + nsz],
                                tps[:msz, :nsz])
                    attnT_list.append(attnT)

                # ---- attention output + bn2 + relu -> h2 (fp8, DoubleRow
                # layout [dp, i, j*n] padded to 400) ----
                h2 = h2p.tile([128, 2, 2, N2P], f8, name=f"h2_{pair}",
                              tag="h2")
                nc.vector.memset(h2[:, :, :, N2:], 0.0)
                for j in range(2):
                    vT = vT_list[j]
                    attnT = attnT_list[j]
                    for c4 in range(PC):
                        aps = ps_sm.tile([128, N], f32, name="aps",
                                         tag="small")
                        for mi, (m0, msz) in enumerate(NCHUNKS):
                            nc.tensor.matmul(
                                aps[:, :],
                                vT[:msz, mi, c4 * 128:(c4 + 1) * 128],
                                attnT[:msz, mi, :],
                                start=(mi == 0), stop=(mi == 1),
                            )
                        nc.scalar.activation(
                            h2[:, c4 // 2, c4 % 2, j * N:(j + 1) * N],
                            aps[:, :], Relu, bias=t2[:, c4:c4 + 1],
                            scale=s2[:, c4:c4 + 1])

                # ---- conv3 (fp8 DoubleRow) + bn3 + residual + relu -> y ----
                ysb = outp.tile([128, KC1, N2], bf16, name=f"y_{pair}",
                                tag="ysb")
                for oc in range(KC1):
                    ops = ps_mm.tile([128, 512], f32, name="ops", tag="mm")
                    for dp in range(2):
                        nc.tensor.matmul(
                            ops[:, :N2],
                            w3t[:, dp, :, oc * 128:(oc + 1) * 128],
                            h2[:, dp, :, :N2],
                            start=(dp == 0), stop=False,
                            perf_mode=DR,
                        )
                    xtile, xi = xparts_all[pair][oc]
                    # accumulate the residual 32*x into PSUM on the PE, so
                    # the epilogue is a single activation (no vector op)
                    nc.tensor.matmul(
                        ops[:, :N2], ident32, xtile[:, xi, :],
                        start=False, stop=True, skip_group_check=True,
                    )
                    nc.scalar.activation(ysb[:, oc, :], ops[:, :N2], Relu,
                                         scale=1.0 / 32,
                                         bias=t3[:, oc:oc + 1])
                    if oc % 2 == 1:
                        g = oc - 1
                        nc.sync.dma_start(
                            out=y_d[:, pair, g * N2:(g + 2) * N2].rearrange(
                                "p (k n) -> p k n", k=2),
                            in_=ysb[:, g:g + 2, :])

    nc.compile()
    return nc


def _prep_inputs(x, w1, g1, b1, m1, v1, wqkv, rel_h, rel_w,
                 g2, b2, m2, v2, w3, g3, b3, m3, v3):
    f = np.float32
    s1 = (g1 / np.sqrt(v1 + EPS)).astype(f)
    t1 = (b1 - m1 * s1).astype(f)
    s2 = (g2 / np.sqrt(v2 + EPS)).astype(f)
    t2 = (b2 - m2 * s2).astype(f)
    s3 = (g3 / np.sqrt(v3 + EPS)).astype(f)
    t3 = (b3 - m3 * s3).astype(f)

    w1p = (w1 * s1[:, None]).astype(f)                    # [512, 2048]
    w1t = np.ascontiguousarray(
        w1p.T.reshape(KC1, 128, P).transpose(1, 0, 2)).astype(BF16)
    # Fold q/k projections: cc = h1^T (Wq^T Wk) h1 -> g = MT^T h1 with
    # MT[e, d] = (Wk^T Wq)[e, d]; cp = pos^T q -> P2^T h1, P2 = Wq^T pos.
    wq = wqkv[:P].astype(np.float64)                      # [512, 512]
    wk = wqkv[P:2 * P].astype(np.float64)
    # fp8 weights are host-scaled by powers of 2 out of the subnormal
    # range; the inverse scale is folded into on-chip copies.
    # DoubleRow layout: contraction row = dp*256 + i*128 + p.
    mtm = (wk.T @ wq) * 64                                # [512e, 512d]
    mt = np.ascontiguousarray(
        mtm.reshape(2, 2, 128, P).transpose(2, 0, 1, 3)).reshape(
        128, 2 * 2 * P).astype(F8E4)
    wv = wqkv[2 * P:].astype(np.float64) * 32             # [512, 512]
    wvt = np.ascontiguousarray(
        wv.T.reshape(2, 2, 128, P).transpose(2, 0, 1, 3)).reshape(
        128, 2 * 2 * P).astype(F8E4)
    w3p = (w3 * s3[:, None]).astype(f) * 32               # [2048, 512]
    w3t = np.ascontiguousarray(
        w3p.T.reshape(2, 2, 128, CIN).transpose(2, 0, 1, 3)).reshape(
        128, 2 * 2 * CIN).astype(F8E4)
    posm = (np.asarray(rel_h, np.float64)
            + np.asarray(rel_w, np.float64)).reshape(P, N)
    p2 = np.asarray(wqkv[:P], np.float64).T @ posm        # [512d, 196n]
    pos = np.ascontiguousarray(
        p2.reshape(PC, 128, N).transpose(1, 0, 2)).astype(BF16)

    t1_h = np.ascontiguousarray(t1.reshape(PC, 128).T)
    s2_h = np.ascontiguousarray(s2.reshape(PC, 128).T)
    t2_h = np.ascontiguousarray(t2.reshape(PC, 128).T)
    t3_h = np.ascontiguousarray(t3.reshape(KC1, 128).T)

    shared = dict(w1t=w1t, mt=mt, wvt=wvt, w3t=w3t, pos=pos,
                  t1=t1_h, s2=s2_h, t2=t2_h, t3=t3_h)

    xb = np.asarray(x, f).astype(BF16)
    in_maps = []
    for c in range(NCORES):
        # [BPC, CIN, H, W] -> [128, NPAIR, KC1 * N2] with free layout
        # per pair: [kc, j, n]
        xc = xb[c * BPC:(c + 1) * BPC].reshape(NPAIR, 2, KC1, 128, N)
        xc = np.ascontiguousarray(xc.transpose(3, 0, 2, 1, 4)).reshape(
            128, NPAIR, KC1 * N2)
        in_maps.append(dict(shared, x=xc))
    return in_maps


def _run(in_maps, trace=False):
    from concourse.bass_utils import run_bass_kernel_spmd
    if "nc" not in _CACHE:
        _CACHE["nc"] = _build()
    nc = _CACHE["nc"]
    return run_bass_kernel_spmd(nc, in_maps, core_ids=list(range(NCORES)),
                                trace=trace)


def _assemble(results):
    out = np.empty((B, CIN, H, W), np.float32)
    for c in range(NCORES):
        # [128, NPAIR, KC1*N2] -> [BPC, CIN, H, W]
        yc = results[c]["y"].reshape(128, NPAIR, KC1, 2, N).astype(
            np.float32)
        out[c * BPC:(c + 1) * BPC] = yc.transpose(1, 3, 2, 0, 4).reshape(
            BPC, CIN, H, W)
    return out


def kernel(**inputs):
    in_maps = _prep_inputs(**inputs)
    res = _run(in_maps)
    return _assemble(res.results)
